# revision 14
# baseline (speedup 1.0000x reference)
"""Trainium2 Bass kernel for LlamaMultiheadLatentAttention.

Contract: kernel(**inputs) takes FULL fp32 inputs (as produced by
reference.setup_inputs) and returns the FULL fp32 output [2, 1024, 4096].

Sharding (8 cores, no collectives): core c handles batch b = c//4 and
head-group g = c%4 (8 query heads, 2 kv heads, 8 latent heads). q/k/v and
latent projections are column-sharded per head-group; o_proj/latent_o_proj
are row-sharded, so each core emits a partial output sum and the host adds
the 4 partials per batch (the "all-reduce" of the output happens at unshard
time on the host).

v2 layout/scheduling choices (on top of the v1 feature-major design):
  - w_lq @ w_lk premultiplied on the host: lk = x @ W2 directly, which
    shards the latent-q work that v1 duplicated across head-group cores.
  - causal trimming at 128-column granularity: score/exp/PV/denominator
    work below the diagonal j-block is skipped (PSUM accumulation into
    column subranges; the start=True matmul of each chain is full-width).
  - softmax denominator via a ones[128,128] stationary matmul, so the
    denominator lands broadcast across all 128 PSUM partitions and the
    normalization is a plain reciprocal+multiply (no partition broadcast).
  - attention is software-pipelined one unit (vh, ib) ahead: the PE does
    scores of unit u+1 between PV and denominator of unit u, giving the
    scalar engine (exp) a full unit of lead time.
  - the output projection is chopped into (col-block, token-tile) chunks;
    chunks for token tiles 0..3 are woven between attention units of the
    second token half as pure-PE filler, the rest run as a tail.
"""

import numpy as np
import ml_dtypes

import concourse.bass as bass
import concourse.mybir as mybir
import concourse.tile as tile
from concourse import bacc
from concourse.bass_utils import run_bass_kernel_spmd

BF16 = ml_dtypes.bfloat16

B, S, D = 2, 1024, 4096
H, KVH, HD = 32, 8, 128
GROUPS = H // KVH
LAT, LH = 1024, 32
THETA = 10000.0
SCALE = 1.0 / float(np.sqrt(HD))

NCORES = 8
TP = 4                 # head-group shards
HL = H // TP           # 8 local q heads
KVL = KVH // TP        # 2 local kv heads
LHL = LH // TP         # 8 local latent heads

f32 = mybir.dt.float32
bf16 = mybir.dt.bfloat16

D_T = D // 128         # 32 k-tiles over model dim
S_T = S // 128         # 8 token tiles of 128
IB = 2                 # token blocks of 512
NB = D // 512          # 8 output column blocks


def _build_program():
    nc = bacc.Bacc("TRN2", target_bir_lowering=False, debug=False)

    xt_d = nc.dram_tensor("xt", [128, D_T, S], bf16, kind="ExternalInput")
    wq_d = nc.dram_tensor("wq", [HL, 128, D_T, 128], bf16, kind="ExternalInput")
    wk_d = nc.dram_tensor("wk", [KVL, 128, D_T, 128], bf16, kind="ExternalInput")
    wv_d = nc.dram_tensor("wv", [128, D_T, KVL * HD], bf16, kind="ExternalInput")
    w2_d = nc.dram_tensor("w2", [LHL, 128, D_T, 128], bf16, kind="ExternalInput")
    wlv_d = nc.dram_tensor("wlv", [128, D_T, LHL * HD], bf16, kind="ExternalInput")
    wo_d = nc.dram_tensor("wo", [NB, 128, HL, 512], bf16, kind="ExternalInput")
    wlo_d = nc.dram_tensor("wlo", [NB, 128, LHL, 512], bf16, kind="ExternalInput")
    cos_d = nc.dram_tensor("cosT", [HD, S], f32, kind="ExternalInput")
    sin_d = nc.dram_tensor("sinTs", [HD, S], f32, kind="ExternalInput")
    mask_d = nc.dram_tensor("maskD", [128, 128], bf16, kind="ExternalInput")
    out_d = nc.dram_tensor("out", [S, D], f32, kind="ExternalOutput")

    out_ap = out_d.ap().rearrange("(tt p) d -> p tt d", p=128)

    with tile.TileContext(nc) as tc:
        with tc.tile_pool(name="const", bufs=1) as constp, \
             tc.tile_pool(name="acts", bufs=1) as acts:

            # persistent activations (bf16); q/k/lk tiles allocated at B1
            v_sb = acts.tile([128, S_T, KVL * HD], bf16, tag="v")
            lv_sb = acts.tile([128, S_T, LHL * HD], bf16, tag="lv")

            with tc.tile_pool(name="xt", bufs=1) as xtp:
                xt = xtp.tile([128, D_T, S], bf16, tag="xt")

                # ---- phase B2: token-major projections v, lv ----
                # wlv streamed in two 512-col halves (double-buffered);
                # first-needed DMA chunks issued first, kt-quarter grain.
                with tc.tile_pool(name="wv", bufs=1) as wvp, \
                     tc.tile_pool(name="wlvh", bufs=2) as wlvp, \
                     tc.tile_pool(name="ps_b2", bufs=4, space="PSUM") as psb2, \
                     tc.tile_pool(name="ps_b2v", bufs=2, space="PSUM") as psb2v:
                    wv_sb = wvp.tile([128, D_T, KVL * HD], bf16, tag="wv")
                    wlv_h = [wlvp.tile([128, D_T, 512], bf16, tag="wlvh",
                                       name=f"wlvh_{q}") for q in range(2)]

                    def dma_h(q, c):
                        nc.sync.dma_start(
                            wlv_h[q][:, bass.ts(c, D_T // 8), :],
                            wlv_d.ap()[:, bass.ts(c, D_T // 8),
                                       bass.ds(q * 512, 512)])

                    # interleaved fine-grained input DMAs, first-needed first
                    for c in range(8):
                        nc.sync.dma_start(
                            xt[:, bass.ts(c, D_T // 8), :],
                            xt_d.ap()[:, bass.ts(c, D_T // 8), :])
                        nc.sync.dma_start(
                            wv_sb[:, bass.ts(c, D_T // 8), :],
                            wv_d.ap()[:, bass.ts(c, D_T // 8), :])
                        dma_h(0, c)
                    for c in range(8):
                        dma_h(1, c)

                    for q in range(2):
                        for tt in range(S_T):
                            ps_lv = psb2.tile([128, 512], f32, tag="ps_lv",
                                              name=f"ps_lv_{q}_{tt}")
                            ps_v = None
                            if q == 0:
                                ps_v = psb2v.tile(
                                    [128, 512], f32, tag="ps_v",
                                    name=f"ps_v_{tt}")
                            for kt in range(D_T):
                                st = kt == 0
                                sp = kt == D_T - 1
                                lhs = xt[:, kt, bass.ts(tt, 128)]
                                nc.tensor.matmul(ps_lv[:], lhs,
                                                 wlv_h[q][:, kt, :],
                                                 start=st, stop=sp)
                                if q == 0:
                                    nc.tensor.matmul(ps_v[:, 0:256], lhs,
                                                     wv_sb[:, kt, :],
                                                     start=st, stop=sp)
                            nc.any.tensor_copy(
                                lv_sb[:, tt, bass.ds(q * 512, 512)],
                                ps_lv[:])
                            if q == 0:
                                nc.any.tensor_copy(v_sb[:, tt, :],
                                                   ps_v[:, 0:256])

                # ---- phase B1: feature-major projections q, k, lk (+rope) --
                qT = acts.tile([128, HL, S], bf16, tag="qT")
                kT = acts.tile([128, KVL, S], bf16, tag="kT")
                lkT = acts.tile([128, LHL, S], bf16, tag="lkT")
                with tc.tile_pool(name="wstr", bufs=3) as wstr, \
                     tc.tile_pool(name="rope", bufs=4) as ropep, \
                     tc.tile_pool(name="const1", bufs=1) as constp1, \
                     tc.tile_pool(name="ps_b1", bufs=4, space="PSUM") as psb1:
                    cosT = constp1.tile([HD, S], f32, tag="cosT")
                    sinTs = constp1.tile([HD, S], f32, tag="sinTs")
                    nc.sync.dma_start(cosT[:], cos_d.ap())
                    nc.sync.dma_start(sinTs[:], sin_d.ap())
                    _uidb = [0]

                    def uname_b1(pfx):
                        _uidb[0] += 1
                        return f"{pfx}_{_uidb[0]}"

                    def rope_to(dst, ps, ib):
                        sl = bass.ts(ib, 512)
                        rt = ropep.tile([128, 512], f32, tag="rt", name=uname_b1("rt"))
                        qc = ropep.tile([128, 512], f32, tag="qc", name=uname_b1("qc"))
                        nc.vector.tensor_tensor(
                            rt[0:64, :], ps[64:128, :], sinTs[0:64, sl],
                            mybir.AluOpType.mult)
                        nc.vector.tensor_tensor(
                            rt[64:128, :], ps[0:64, :], sinTs[64:128, sl],
                            mybir.AluOpType.mult)
                        nc.vector.tensor_tensor(
                            qc[:], ps[:], cosT[:, sl], mybir.AluOpType.mult)
                        nc.vector.tensor_add(dst, qc[:], rt[:])

                    def proj_fm(w_dram, n_tiles, dst):
                        for nt in range(n_tiles):
                            wt = wstr.tile([128, D_T, 128], bf16, tag="w_fm", name=uname_b1("w_fm"))
                            nc.sync.dma_start(wt[:], w_dram.ap()[nt])
                            ps = [psb1.tile([128, 512], f32, tag="ps_b1",
                                            name=f"ps_b1_{nt}_{ib}")
                                  for ib in range(IB)]
                            for kt in range(D_T):
                                for ib in range(IB):
                                    nc.tensor.matmul(
                                        ps[ib][:], wt[:, kt, :],
                                        xt[:, kt, bass.ts(ib, 512)],
                                        start=(kt == 0), stop=(kt == D_T - 1))
                            for ib in range(IB):
                                rope_to(dst[:, nt, bass.ts(ib, 512)],
                                        ps[ib][:], ib)

                    proj_fm(wq_d, HL, qT)
                    proj_fm(wk_d, KVL, kT)
                    proj_fm(w2_d, LHL, lkT)

            # ---- phase C+D: attention with output-projection weave ----
            with tc.tile_pool(name="attnlat", bufs=1) as alp:
                attnT = alp.tile([128, HL, S], bf16, tag="attnT")
                latT = alp.tile([128, LHL, S], bf16, tag="latT")

                with tc.tile_pool(name="pp", bufs=16) as pp, \
                     tc.tile_pool(name="dn", bufs=3) as dn, \
                     tc.tile_pool(name="wop", bufs=3) as wop, \
                     tc.tile_pool(name="ost", bufs=6) as ost, \
                     tc.tile_pool(name="const2", bufs=1) as constp2:
                    maskD = constp2.tile([128, 128], bf16, tag="maskD")
                    ones = constp2.tile([128, 128], bf16, tag="ones")
                    nc.sync.dma_start(maskD[:], mask_d.ap())
                    nc.vector.memset(ones[:], 1.0)
                    wo_seq = [0]
                    _uid = [0]

                    def uname(pfx):
                        _uid[0] += 1
                        return f"{pfx}_{_uid[0]}"

                    def vh_parts(vh):
                        if vh < HL:
                            h = vh
                            return (h, kT[:, h // GROUPS, :], attnT,
                                    lambda jb: v_sb[:, jb,
                                                    bass.ts(h // GROUPS, HD)])
                        h = vh - HL
                        return (h, lkT[:, h, :], latT,
                                lambda jb: lv_sb[:, jb, bass.ts(h, HD)])

                    def wo_dma(nb):
                        sq = wo_seq[0]
                        wo_seq[0] += 1
                        w = wop.tile([128, HL, 512], bf16, tag="wo",
                                     name=f"wo_{sq}")
                        wl = wop.tile([128, LHL, 512], bf16, tag="wlo",
                                      name=f"wlo_{sq}")
                        nc.sync.dma_start(w[:], wo_d.ap()[nb])
                        nc.sync.dma_start(wl[:], wlo_d.ap()[nb])
                        return w, wl

                    def d_chunk(psf_pool, wpair, nb, tt):
                        w, wl = wpair
                        psf = psf_pool.tile([128, 512], f32, tag="psf", name=uname("psf"))
                        for h in range(HL):
                            nc.tensor.matmul(
                                psf[:], attnT[:, h, bass.ts(tt, 128)],
                                w[:, h, :], start=(h == 0), stop=False)
                        for h in range(LHL):
                            nc.tensor.matmul(
                                psf[:], latT[:, h, bass.ts(tt, 128)],
                                wl[:, h, :], start=False, stop=(h == LHL - 1))
                        ot = ost.tile([128, 512], f32, tag="ot", name=uname("ot"))
                        nc.any.tensor_copy(ot[:], psf[:])
                        nc.sync.dma_start(
                            out_ap[:, tt, bass.ds(nb * 512, 512)], ot[:])

                    with tc.tile_pool(name="ps_s", bufs=3, space="PSUM") as pss_, \
                         tc.tile_pool(name="ps_o", bufs=2, space="PSUM") as pso_, \
                         tc.tile_pool(name="ps_d", bufs=1, space="PSUM") as psd_, \
                         tc.tile_pool(name="ps_f", bufs=2, space="PSUM") as psf_:

                        def off_of(jb, ib):
                            return max(jb - 4 * ib, 0) * 128

                        def emit_scores(u, half):
                            # scores + exp + mask for jbs of one half
                            vh, ib, pts = u
                            h, ksrc, _, _ = vh_parts(vh)
                            njb = 4 * (ib + 1)
                            lo = 0 if half == 0 else njb // 2
                            hi = njb // 2 if half == 0 else njb
                            for jb in range(lo, hi):
                                off = off_of(jb, ib)
                                ps_s = pss_.tile([128, 512], f32, tag="ps_s", name=uname("ps_s"))
                                nc.tensor.matmul(
                                    ps_s[:, off:512],
                                    ksrc[:, bass.ts(jb, 128)],
                                    qT[:, h, bass.ds(ib * 512 + off,
                                                     512 - off)],
                                    start=True, stop=True)
                                pt = pp.tile([128, 512], bf16, tag="pt", name=uname("pt"))
                                nc.scalar.activation(
                                    pt[:, off:512], ps_s[:, off:512],
                                    mybir.ActivationFunctionType.Exp,
                                    scale=SCALE)
                                if jb >= 4 * ib:
                                    nc.vector.tensor_tensor(
                                        pt[:, off:off + 128],
                                        pt[:, off:off + 128], maskD[:],
                                        mybir.AluOpType.mult)
                                pts.append(pt)

                        def emit_pv(u):
                            vh, ib, pts = u
                            _, _, _, vsl = vh_parts(vh)
                            njb = 4 * (ib + 1)
                            u.append(pso_.tile([128, 512], f32, tag="ps_o", name=uname("ps_o")))
                            ps_o = u[3]
                            for jb in range(njb):
                                off = off_of(jb, ib)
                                nc.tensor.matmul(
                                    ps_o[:, off:512], vsl(jb),
                                    pts[jb][:, off:512],
                                    start=(jb == 0), stop=(jb == njb - 1))

                        def emit_den_norm(u):
                            vh, ib, pts, ps_o = u
                            h, _, dst, _ = vh_parts(vh)
                            njb = 4 * (ib + 1)
                            ps_d = psd_.tile([128, 512], f32, tag="ps_d", name=uname("ps_d"))
                            for jb in range(njb):
                                off = off_of(jb, ib)
                                nc.tensor.matmul(
                                    ps_d[:, off:512], ones[:],
                                    pts[jb][:, off:512],
                                    start=(jb == 0), stop=(jb == njb - 1))
                            rec = dn.tile([128, 512], f32, tag="rec", name=uname("rec"))
                            nc.vector.reciprocal(rec[:], ps_d[:])
                            nc.vector.tensor_tensor(
                                dst[:, h, bass.ts(ib, 512)], ps_o[:], rec[:],
                                mybir.AluOpType.mult)

                        # --- pass ib=0: software-pipelined, one unit of lag
                        wpairs = {0: wo_dma(0), 1: wo_dma(1)}
                        prev = None
                        for vh in range(HL + LHL):
                            u = [vh, 0, []]
                            emit_scores(u, 0)
                            emit_scores(u, 1)
                            if prev is not None:
                                emit_pv(prev)
                                emit_den_norm(prev)
                            prev = u

                        # --- pass ib=1 with output-projection chunks woven
                        chunks = [(nb, tt) for nb in range(NB)
                                  for tt in range(S_T // 2)]
                        ci = 0
                        for vh in range(HL + LHL):
                            u = [vh, 1, []]
                            emit_scores(u, 0)
                            emit_pv(prev)
                            emit_scores(u, 1)
                            emit_den_norm(prev)
                            prev = u
                            for _ in range(2):
                                if ci < len(chunks):
                                    nb, tt = chunks[ci]
                                    for ahead in (1, 2):
                                        nba = nb + ahead
                                        if (nba < NB and nba not in wpairs
                                                and tt == 2 * ahead - 2):
                                            wpairs[nba] = wo_dma(nba)
                                    d_chunk(psf_, wpairs[nb], nb, tt)
                                    wpairs.pop(nb - 1, None)
                                    ci += 1
                        emit_pv(prev)
                        emit_den_norm(prev)

                    # --- output-projection tail: token tiles 4..7
                    with tc.tile_pool(name="ps_f2", bufs=4,
                                      space="PSUM") as psf2_:
                        tpair = {0: wo_dma(0), 1: wo_dma(1)}
                        for nb in range(NB):
                            for tt in range(S_T // 2, S_T):
                                if tt == S_T // 2 and nb + 2 < NB:
                                    tpair[nb + 2] = wo_dma(nb + 2)
                                d_chunk(psf2_, tpair[nb], nb, tt)
                            tpair.pop(nb, None)

    nc.compile()
    return nc


_NC = None


def _get_program():
    global _NC
    if _NC is None:
        _NC = _build_program()
    return _NC


def _rope_tables():
    inv_freq = 1.0 / (THETA ** (np.arange(0, HD, 2, dtype=np.float32) / HD))
    t = np.arange(S, dtype=np.float32)
    freqs = np.outer(t, inv_freq)                       # [S, 64]
    emb = np.concatenate([freqs, freqs], axis=-1)       # [S, HD]
    cosT = np.cos(emb).T.astype(np.float32).copy()      # [HD, S]
    sinT = np.sin(emb).T.astype(np.float32)
    sinTs = np.concatenate([-sinT[:HD // 2], sinT[HD // 2:]], 0).astype(
        np.float32).copy()
    return cosT, sinTs


def _mask_diag():
    # maskD[p, i] = 1.0 iff p <= i (upper-triangular incl. diagonal)
    p = np.arange(128)[:, None]
    i = np.arange(128)[None, :]
    return (p <= i).astype(BF16)


def _tile_w_fm(w, n_tiles, kt):
    # [K, n_tiles*128] -> [n_tiles, 128(p of K), kt, 128]
    K, N = w.shape
    assert K == kt * 128 and N == n_tiles * 128
    return np.ascontiguousarray(
        w.reshape(kt, 128, n_tiles, 128).transpose(2, 1, 0, 3)).astype(BF16)


def _tile_w_tm(w, kt):
    # [K, N] -> [128(p of K), kt, N]
    K, N = w.shape
    assert K == kt * 128
    return np.ascontiguousarray(
        w.reshape(kt, 128, N).transpose(1, 0, 2)).astype(BF16)


def _tile_w_out(w):
    # [1024, D] -> [8(nb), 128(p of rows), 8(h), 512]
    return np.ascontiguousarray(
        w.reshape(8, 128, D // 512, 512).transpose(2, 1, 0, 3)).astype(BF16)


def _make_in_maps(hidden_states, w_q, w_k, w_v, w_o, w_lq, w_lk, w_lv, w_lo):
    cosT, sinTs = _rope_tables()
    maskD = _mask_diag()
    w2 = np.asarray(w_lq, dtype=np.float32) @ np.asarray(w_lk,
                                                         dtype=np.float32)
    in_maps = []
    for c in range(NCORES):
        b, g = divmod(c, TP)
        x = np.asarray(hidden_states[b], dtype=np.float32)       # [S, D]
        xt = np.ascontiguousarray(
            x.T.reshape(D_T, 128, S).transpose(1, 0, 2)).astype(BF16)
        qs = slice(g * HL * HD, (g + 1) * HL * HD)
        kvs = slice(g * KVL * HD, (g + 1) * KVL * HD)
        ls = slice(g * LHL * HD, (g + 1) * LHL * HD)
        in_maps.append({
            "xt": xt,
            "wq": _tile_w_fm(np.asarray(w_q)[:, qs], HL, D_T),
            "wk": _tile_w_fm(np.asarray(w_k)[:, kvs], KVL, D_T),
            "wv": _tile_w_tm(np.asarray(w_v)[:, kvs], D_T),
            "w2": _tile_w_fm(w2[:, ls], LHL, D_T),
            "wlv": _tile_w_tm(np.asarray(w_lv)[:, ls], D_T),
            "wo": _tile_w_out(np.asarray(w_o)[qs, :]),
            "wlo": _tile_w_out(np.asarray(w_lo)[ls, :]),
            "cosT": cosT,
            "sinTs": sinTs,
            "maskD": maskD,
        })
    return in_maps


def kernel(hidden_states, w_q, w_k, w_v, w_o, w_lq, w_lk, w_lv, w_lo):
    nc = _get_program()
    in_maps = _make_in_maps(hidden_states, w_q, w_k, w_v, w_o,
                            w_lq, w_lk, w_lv, w_lo)
    res = run_bass_kernel_spmd(nc, in_maps, list(range(NCORES))).results

    out = np.zeros((B, S, D), dtype=np.float32)
    for c in range(NCORES):
        b = c // TP
        out[b] += res[c]["out"]
    return out


# revision 16
# speedup vs baseline: 1.0025x; 1.0025x over previous
"""Trainium2 Bass kernel for LlamaMultiheadLatentAttention.

Contract: kernel(**inputs) takes FULL fp32 inputs (as produced by
reference.setup_inputs) and returns the FULL fp32 output [2, 1024, 4096].

Sharding (8 cores, no collectives): core c handles batch b = c//4 and
head-group g = c%4 (8 query heads, 2 kv heads, 8 latent heads). q/k/v and
latent projections are column-sharded per head-group; o_proj/latent_o_proj
are row-sharded, so each core emits a partial output sum and the host adds
the 4 partials per batch (the "all-reduce" of the output happens at unshard
time on the host).

v2 layout/scheduling choices (on top of the v1 feature-major design):
  - w_lq @ w_lk premultiplied on the host: lk = x @ W2 directly, which
    shards the latent-q work that v1 duplicated across head-group cores.
  - causal trimming at 128-column granularity: score/exp/PV/denominator
    work below the diagonal j-block is skipped (PSUM accumulation into
    column subranges; the start=True matmul of each chain is full-width).
  - softmax denominator via a ones[128,128] stationary matmul, so the
    denominator lands broadcast across all 128 PSUM partitions and the
    normalization is a plain reciprocal+multiply (no partition broadcast).
  - attention is software-pipelined one unit (vh, ib) ahead: the PE does
    scores of unit u+1 between PV and denominator of unit u, giving the
    scalar engine (exp) a full unit of lead time.
  - the output projection is chopped into (col-block, token-tile) chunks;
    chunks for token tiles 0..3 are woven between attention units of the
    second token half as pure-PE filler, the rest run as a tail.
"""

import numpy as np
import ml_dtypes

import concourse.bass as bass
import concourse.mybir as mybir
import concourse.tile as tile
from concourse import bacc
from concourse.bass_utils import run_bass_kernel_spmd

BF16 = ml_dtypes.bfloat16

B, S, D = 2, 1024, 4096
H, KVH, HD = 32, 8, 128
GROUPS = H // KVH
LAT, LH = 1024, 32
THETA = 10000.0
SCALE = 1.0 / float(np.sqrt(HD))

NCORES = 8
TP = 4                 # head-group shards
HL = H // TP           # 8 local q heads
KVL = KVH // TP        # 2 local kv heads
LHL = LH // TP         # 8 local latent heads

f32 = mybir.dt.float32
bf16 = mybir.dt.bfloat16

D_T = D // 128         # 32 k-tiles over model dim
S_T = S // 128         # 8 token tiles of 128
IB = 2                 # token blocks of 512
NB = D // 512          # 8 output column blocks


def _build_program():
    nc = bacc.Bacc("TRN2", target_bir_lowering=False, debug=False)

    xt_d = nc.dram_tensor("xt", [128, D_T, S], bf16, kind="ExternalInput")
    wq_d = nc.dram_tensor("wq", [HL, 128, D_T, 128], bf16, kind="ExternalInput")
    wk_d = nc.dram_tensor("wk", [KVL, 128, D_T, 128], bf16, kind="ExternalInput")
    wv_d = nc.dram_tensor("wv", [128, D_T, KVL * HD], bf16, kind="ExternalInput")
    w2_d = nc.dram_tensor("w2", [LHL, 128, D_T, 128], bf16, kind="ExternalInput")
    wlv_d = nc.dram_tensor("wlv", [128, D_T, LHL * HD], bf16, kind="ExternalInput")
    wo_d = nc.dram_tensor("wo", [NB, 128, HL, 512], bf16, kind="ExternalInput")
    wlo_d = nc.dram_tensor("wlo", [NB, 128, LHL, 512], bf16, kind="ExternalInput")
    cos_d = nc.dram_tensor("cosT", [HD, S], f32, kind="ExternalInput")
    sin_d = nc.dram_tensor("sinTs", [HD, S], f32, kind="ExternalInput")
    mask_d = nc.dram_tensor("maskD", [128, 128], bf16, kind="ExternalInput")
    out_d = nc.dram_tensor("out", [S, D], f32, kind="ExternalOutput")

    out_ap = out_d.ap().rearrange("(tt p) d -> p tt d", p=128)

    with tile.TileContext(nc) as tc:
        with tc.tile_pool(name="const", bufs=1) as constp, \
             tc.tile_pool(name="acts", bufs=1) as acts:

            # persistent activations (bf16); q/k/lk tiles allocated at B1
            v_sb = acts.tile([128, S_T, KVL * HD], bf16, tag="v")
            lv_sb = acts.tile([128, S_T, LHL * HD], bf16, tag="lv")

            with tc.tile_pool(name="xt", bufs=1) as xtp:
                xt = xtp.tile([128, D_T, S], bf16, tag="xt")

                # preload zone: B1's first weight tile + rope tables live in
                # memory that is never recycled from B2's streaming pools, so
                # their DMAs run during B2 compute instead of after it.
                wpre = xtp.tile([128, D_T, 128], bf16, tag="wpre")
                cosT = xtp.tile([HD, S], f32, tag="cosT")
                sinTs = xtp.tile([HD, S], f32, tag="sinTs")

                # ---- phase B2: token-major projections v, lv ----
                # wlv streamed in four 256-col quarters (double-buffered);
                # first-needed DMA chunks issued first, kt-quarter grain.
                with tc.tile_pool(name="wv", bufs=1) as wvp, \
                     tc.tile_pool(name="wlvq", bufs=2) as wlvp, \
                     tc.tile_pool(name="ps_b2", bufs=4, space="PSUM") as psb2, \
                     tc.tile_pool(name="ps_b2v", bufs=2, space="PSUM") as psb2v:
                    wv_sb = wvp.tile([128, D_T, KVL * HD], bf16, tag="wv")
                    wlv_q = [wlvp.tile([128, D_T, 256], bf16, tag="wlvq",
                                       name=f"wlvq_{q}") for q in range(4)]

                    def dma_q(q, c):
                        nc.sync.dma_start(
                            wlv_q[q][:, bass.ts(c, D_T // 8), :],
                            wlv_d.ap()[:, bass.ts(c, D_T // 8),
                                       bass.ds(q * 256, 256)])

                    # interleaved fine-grained input DMAs, first-needed first
                    for c in range(8):
                        nc.sync.dma_start(
                            xt[:, bass.ts(c, D_T // 8), :],
                            xt_d.ap()[:, bass.ts(c, D_T // 8), :])
                        nc.sync.dma_start(
                            wv_sb[:, bass.ts(c, D_T // 8), :],
                            wv_d.ap()[:, bass.ts(c, D_T // 8), :])
                        dma_q(0, c)
                    for c in range(8):
                        dma_q(1, c)
                    nc.sync.dma_start(wpre[:], wq_d.ap()[0])
                    nc.sync.dma_start(cosT[:], cos_d.ap())
                    nc.sync.dma_start(sinTs[:], sin_d.ap())

                    for q in range(4):
                        if q < 2:
                            for c in range(8):
                                dma_q(q + 2, c)
                        for tt in range(S_T):
                            ps_lv = psb2.tile([128, 512], f32, tag="ps_lv",
                                              name=f"ps_lv_{q}_{tt}")
                            ps_v = None
                            if q == 0:
                                ps_v = psb2v.tile(
                                    [128, 512], f32, tag="ps_v",
                                    name=f"ps_v_{tt}")
                            for kt in range(D_T):
                                st = kt == 0
                                sp = kt == D_T - 1
                                lhs = xt[:, kt, bass.ts(tt, 128)]
                                nc.tensor.matmul(ps_lv[:, 0:256], lhs,
                                                 wlv_q[q][:, kt, :],
                                                 start=st, stop=sp)
                                if q == 0:
                                    nc.tensor.matmul(ps_v[:, 0:256], lhs,
                                                     wv_sb[:, kt, :],
                                                     start=st, stop=sp)
                            nc.any.tensor_copy(
                                lv_sb[:, tt, bass.ds(q * 256, 256)],
                                ps_lv[:, 0:256])
                            if q == 0:
                                nc.any.tensor_copy(v_sb[:, tt, :],
                                                   ps_v[:, 0:256])

                # ---- phase B1: feature-major projections q, k, lk (+rope) --
                qT = acts.tile([128, HL, S], bf16, tag="qT")
                kT = acts.tile([128, KVL, S], bf16, tag="kT")
                lkT = acts.tile([128, LHL, S], bf16, tag="lkT")
                with tc.tile_pool(name="wstr", bufs=3) as wstr, \
                     tc.tile_pool(name="rope", bufs=4) as ropep, \
                     tc.tile_pool(name="ps_b1", bufs=4, space="PSUM") as psb1:
                    _uidb = [0]

                    def uname_b1(pfx):
                        _uidb[0] += 1
                        return f"{pfx}_{_uidb[0]}"

                    def rope_to(dst, ps, ib):
                        sl = bass.ts(ib, 512)
                        rt = ropep.tile([128, 512], f32, tag="rt", name=uname_b1("rt"))
                        qc = ropep.tile([128, 512], f32, tag="qc", name=uname_b1("qc"))
                        nc.vector.tensor_tensor(
                            rt[0:64, :], ps[64:128, :], sinTs[0:64, sl],
                            mybir.AluOpType.mult)
                        nc.vector.tensor_tensor(
                            rt[64:128, :], ps[0:64, :], sinTs[64:128, sl],
                            mybir.AluOpType.mult)
                        nc.vector.tensor_tensor(
                            qc[:], ps[:], cosT[:, sl], mybir.AluOpType.mult)
                        nc.vector.tensor_add(dst, qc[:], rt[:])

                    def proj_fm(w_dram, n_tiles, dst, pre=None):
                        for nt in range(n_tiles):
                            if nt == 0 and pre is not None:
                                wt = pre
                            else:
                                wt = wstr.tile([128, D_T, 128], bf16,
                                               tag="w_fm", name=uname_b1("w_fm"))
                                nc.sync.dma_start(wt[:], w_dram.ap()[nt])
                            ps = [psb1.tile([128, 512], f32, tag="ps_b1",
                                            name=f"ps_b1_{nt}_{ib}")
                                  for ib in range(IB)]
                            for kt in range(D_T):
                                for ib in range(IB):
                                    nc.tensor.matmul(
                                        ps[ib][:], wt[:, kt, :],
                                        xt[:, kt, bass.ts(ib, 512)],
                                        start=(kt == 0), stop=(kt == D_T - 1))
                            for ib in range(IB):
                                rope_to(dst[:, nt, bass.ts(ib, 512)],
                                        ps[ib][:], ib)

                    proj_fm(wq_d, HL, qT, pre=wpre)
                    proj_fm(wk_d, KVL, kT)
                    proj_fm(w2_d, LHL, lkT)

            # ---- phase C+D: attention with output-projection weave ----
            with tc.tile_pool(name="attnlat", bufs=1) as alp:
                attnT = alp.tile([128, HL, S], bf16, tag="attnT")
                latT = alp.tile([128, LHL, S], bf16, tag="latT")

                with tc.tile_pool(name="pp", bufs=16) as pp, \
                     tc.tile_pool(name="dn", bufs=3) as dn, \
                     tc.tile_pool(name="wop", bufs=4) as wop, \
                     tc.tile_pool(name="ost", bufs=6) as ost, \
                     tc.tile_pool(name="const2", bufs=1) as constp2:
                    maskD = constp2.tile([128, 128], bf16, tag="maskD")
                    ones = constp2.tile([128, 128], bf16, tag="ones")
                    nc.sync.dma_start(maskD[:], mask_d.ap())
                    nc.vector.memset(ones[:], 1.0)
                    wo_seq = [0]
                    _uid = [0]

                    def uname(pfx):
                        _uid[0] += 1
                        return f"{pfx}_{_uid[0]}"

                    def vh_parts(vh):
                        if vh < HL:
                            h = vh
                            return (h, kT[:, h // GROUPS, :], attnT,
                                    lambda jb: v_sb[:, jb,
                                                    bass.ts(h // GROUPS, HD)])
                        h = vh - HL
                        return (h, lkT[:, h, :], latT,
                                lambda jb: lv_sb[:, jb, bass.ts(h, HD)])

                    def wo_dma(nb):
                        sq = wo_seq[0]
                        wo_seq[0] += 1
                        w = wop.tile([128, HL, 512], bf16, tag="wo",
                                     name=f"wo_{sq}")
                        wl = wop.tile([128, LHL, 512], bf16, tag="wlo",
                                      name=f"wlo_{sq}")
                        nc.sync.dma_start(w[:], wo_d.ap()[nb])
                        nc.sync.dma_start(wl[:], wlo_d.ap()[nb])
                        return w, wl

                    def d_chunk(psf_pool, wpair, nb, tt):
                        w, wl = wpair
                        psf = psf_pool.tile([128, 512], f32, tag="psf", name=uname("psf"))
                        for h in range(HL):
                            nc.tensor.matmul(
                                psf[:], attnT[:, h, bass.ts(tt, 128)],
                                w[:, h, :], start=(h == 0), stop=False)
                        for h in range(LHL):
                            nc.tensor.matmul(
                                psf[:], latT[:, h, bass.ts(tt, 128)],
                                wl[:, h, :], start=False, stop=(h == LHL - 1))
                        ot = ost.tile([128, 512], f32, tag="ot", name=uname("ot"))
                        nc.vector.tensor_copy(ot[:], psf[:])
                        nc.sync.dma_start(
                            out_ap[:, tt, bass.ds(nb * 512, 512)], ot[:])

                    with tc.tile_pool(name="ps_s", bufs=3, space="PSUM") as pss_, \
                         tc.tile_pool(name="ps_o", bufs=2, space="PSUM") as pso_, \
                         tc.tile_pool(name="ps_d", bufs=1, space="PSUM") as psd_, \
                         tc.tile_pool(name="ps_f", bufs=2, space="PSUM") as psf_:

                        def off_of(jb, ib):
                            return max(jb - 4 * ib, 0) * 128

                        def emit_scores(u, half):
                            # scores + exp + mask for jbs of one half
                            vh, ib, pts = u
                            h, ksrc, _, _ = vh_parts(vh)
                            njb = 4 * (ib + 1)
                            lo = 0 if half == 0 else njb // 2
                            hi = njb // 2 if half == 0 else njb
                            for jb in range(lo, hi):
                                off = off_of(jb, ib)
                                ps_s = pss_.tile([128, 512], f32, tag="ps_s", name=uname("ps_s"))
                                nc.tensor.matmul(
                                    ps_s[:, off:512],
                                    ksrc[:, bass.ts(jb, 128)],
                                    qT[:, h, bass.ds(ib * 512 + off,
                                                     512 - off)],
                                    start=True, stop=True)
                                pt = pp.tile([128, 512], bf16, tag="pt", name=uname("pt"))
                                nc.scalar.activation(
                                    pt[:, off:512], ps_s[:, off:512],
                                    mybir.ActivationFunctionType.Exp,
                                    scale=SCALE)
                                if jb >= 4 * ib:
                                    nc.vector.tensor_tensor(
                                        pt[:, off:off + 128],
                                        pt[:, off:off + 128], maskD[:],
                                        mybir.AluOpType.mult)
                                pts.append(pt)

                        def emit_pv(u):
                            vh, ib, pts = u
                            _, _, _, vsl = vh_parts(vh)
                            njb = 4 * (ib + 1)
                            u.append(pso_.tile([128, 512], f32, tag="ps_o", name=uname("ps_o")))
                            ps_o = u[3]
                            for jb in range(njb):
                                off = off_of(jb, ib)
                                nc.tensor.matmul(
                                    ps_o[:, off:512], vsl(jb),
                                    pts[jb][:, off:512],
                                    start=(jb == 0), stop=(jb == njb - 1))

                        def emit_den_norm(u):
                            vh, ib, pts, ps_o = u
                            h, _, dst, _ = vh_parts(vh)
                            njb = 4 * (ib + 1)
                            ps_d = psd_.tile([128, 512], f32, tag="ps_d", name=uname("ps_d"))
                            for jb in range(njb):
                                off = off_of(jb, ib)
                                nc.tensor.matmul(
                                    ps_d[:, off:512], ones[:],
                                    pts[jb][:, off:512],
                                    start=(jb == 0), stop=(jb == njb - 1))
                            rec = dn.tile([128, 512], f32, tag="rec", name=uname("rec"))
                            nc.vector.reciprocal(rec[:], ps_d[:])
                            nc.vector.tensor_tensor(
                                dst[:, h, bass.ts(ib, 512)], ps_o[:], rec[:],
                                mybir.AluOpType.mult)

                        # --- pass ib=0: software-pipelined, one unit of lag
                        wpairs = {0: wo_dma(0), 1: wo_dma(1)}
                        prev = None
                        for vh in range(HL + LHL):
                            u = [vh, 0, []]
                            emit_scores(u, 0)
                            emit_scores(u, 1)
                            if prev is not None:
                                emit_pv(prev)
                                emit_den_norm(prev)
                            prev = u

                        # --- pass ib=1 with output-projection chunks woven
                        chunks = [(nb, tt) for nb in range(NB)
                                  for tt in range(S_T // 2)]
                        ci = 0
                        for vh in range(HL + LHL):
                            u = [vh, 1, []]
                            emit_scores(u, 0)
                            emit_pv(prev)
                            emit_scores(u, 1)
                            emit_den_norm(prev)
                            prev = u
                            for _ in range(2):
                                if ci < len(chunks):
                                    nb, tt = chunks[ci]
                                    for ahead in (1, 2):
                                        nba = nb + ahead
                                        if (nba < NB and nba not in wpairs
                                                and tt == 2 * ahead - 2):
                                            wpairs[nba] = wo_dma(nba)
                                    d_chunk(psf_, wpairs[nb], nb, tt)
                                    wpairs.pop(nb - 1, None)
                                    ci += 1
                        emit_pv(prev)
                        emit_den_norm(prev)

                    # --- output-projection tail: token tiles 4..7
                    with tc.tile_pool(name="ps_f2", bufs=4,
                                      space="PSUM") as psf2_:
                        tpair = {0: wo_dma(0), 1: wo_dma(1)}
                        for nb in range(NB):
                            for tt in range(S_T // 2, S_T):
                                if tt == S_T // 2 and nb + 2 < NB:
                                    tpair[nb + 2] = wo_dma(nb + 2)
                                d_chunk(psf2_, tpair[nb], nb, tt)
                            tpair.pop(nb, None)

    nc.compile()
    return nc


_NC = None


def _get_program():
    global _NC
    if _NC is None:
        _NC = _build_program()
    return _NC


def _rope_tables():
    inv_freq = 1.0 / (THETA ** (np.arange(0, HD, 2, dtype=np.float32) / HD))
    t = np.arange(S, dtype=np.float32)
    freqs = np.outer(t, inv_freq)                       # [S, 64]
    emb = np.concatenate([freqs, freqs], axis=-1)       # [S, HD]
    cosT = np.cos(emb).T.astype(np.float32).copy()      # [HD, S]
    sinT = np.sin(emb).T.astype(np.float32)
    sinTs = np.concatenate([-sinT[:HD // 2], sinT[HD // 2:]], 0).astype(
        np.float32).copy()
    return cosT, sinTs


def _mask_diag():
    # maskD[p, i] = 1.0 iff p <= i (upper-triangular incl. diagonal)
    p = np.arange(128)[:, None]
    i = np.arange(128)[None, :]
    return (p <= i).astype(BF16)


def _tile_w_fm(w, n_tiles, kt):
    # [K, n_tiles*128] -> [n_tiles, 128(p of K), kt, 128]
    K, N = w.shape
    assert K == kt * 128 and N == n_tiles * 128
    return np.ascontiguousarray(
        w.reshape(kt, 128, n_tiles, 128).transpose(2, 1, 0, 3)).astype(BF16)


def _tile_w_tm(w, kt):
    # [K, N] -> [128(p of K), kt, N]
    K, N = w.shape
    assert K == kt * 128
    return np.ascontiguousarray(
        w.reshape(kt, 128, N).transpose(1, 0, 2)).astype(BF16)


def _tile_w_out(w):
    # [1024, D] -> [8(nb), 128(p of rows), 8(h), 512]
    return np.ascontiguousarray(
        w.reshape(8, 128, D // 512, 512).transpose(2, 1, 0, 3)).astype(BF16)


def _make_in_maps(hidden_states, w_q, w_k, w_v, w_o, w_lq, w_lk, w_lv, w_lo):
    cosT, sinTs = _rope_tables()
    maskD = _mask_diag()
    w2 = np.asarray(w_lq, dtype=np.float32) @ np.asarray(w_lk,
                                                         dtype=np.float32)
    in_maps = []
    for c in range(NCORES):
        b, g = divmod(c, TP)
        x = np.asarray(hidden_states[b], dtype=np.float32)       # [S, D]
        xt = np.ascontiguousarray(
            x.T.reshape(D_T, 128, S).transpose(1, 0, 2)).astype(BF16)
        qs = slice(g * HL * HD, (g + 1) * HL * HD)
        kvs = slice(g * KVL * HD, (g + 1) * KVL * HD)
        ls = slice(g * LHL * HD, (g + 1) * LHL * HD)
        in_maps.append({
            "xt": xt,
            "wq": _tile_w_fm(np.asarray(w_q)[:, qs], HL, D_T),
            "wk": _tile_w_fm(np.asarray(w_k)[:, kvs], KVL, D_T),
            "wv": _tile_w_tm(np.asarray(w_v)[:, kvs], D_T),
            "w2": _tile_w_fm(w2[:, ls], LHL, D_T),
            "wlv": _tile_w_tm(np.asarray(w_lv)[:, ls], D_T),
            "wo": _tile_w_out(np.asarray(w_o)[qs, :]),
            "wlo": _tile_w_out(np.asarray(w_lo)[ls, :]),
            "cosT": cosT,
            "sinTs": sinTs,
            "maskD": maskD,
        })
    return in_maps


def kernel(hidden_states, w_q, w_k, w_v, w_o, w_lq, w_lk, w_lv, w_lo):
    nc = _get_program()
    in_maps = _make_in_maps(hidden_states, w_q, w_k, w_v, w_o,
                            w_lq, w_lk, w_lv, w_lo)
    res = run_bass_kernel_spmd(nc, in_maps, list(range(NCORES))).results

    out = np.zeros((B, S, D), dtype=np.float32)
    for c in range(NCORES):
        b = c // TP
        out[b] += res[c]["out"]
    return out


# revision 17
# speedup vs baseline: 1.0632x; 1.0606x over previous
"""Trainium2 Bass kernel for LlamaMultiheadLatentAttention.

Contract: kernel(**inputs) takes FULL fp32 inputs (as produced by
reference.setup_inputs) and returns the FULL fp32 output [2, 1024, 4096].

Sharding (8 cores, no collectives): core c handles batch b = c//4 and
head-group g = c%4 (8 query heads, 2 kv heads, 8 latent heads). q/k/v and
latent projections are column-sharded per head-group; o_proj/latent_o_proj
are row-sharded, so each core emits a partial output sum and the host adds
the 4 partials per batch (the "all-reduce" of the output happens at unshard
time on the host).

v2 layout/scheduling choices (on top of the v1 feature-major design):
  - w_lq @ w_lk premultiplied on the host: lk = x @ W2 directly, which
    shards the latent-q work that v1 duplicated across head-group cores.
  - causal trimming at 128-column granularity: score/exp/PV/denominator
    work below the diagonal j-block is skipped (PSUM accumulation into
    column subranges; the start=True matmul of each chain is full-width).
  - softmax denominator via a ones[128,128] stationary matmul, so the
    denominator lands broadcast across all 128 PSUM partitions and the
    normalization is a plain reciprocal+multiply (no partition broadcast).
  - attention is software-pipelined one unit (vh, ib) ahead: the PE does
    scores of unit u+1 between PV and denominator of unit u, giving the
    scalar engine (exp) a full unit of lead time.
  - the output projection is chopped into (col-block, token-tile) chunks;
    chunks for token tiles 0..3 are woven between attention units of the
    second token half as pure-PE filler, the rest run as a tail.
"""

import numpy as np
import ml_dtypes

import concourse.bass as bass
import concourse.mybir as mybir
import concourse.tile as tile
from concourse import bacc
from concourse.bass_utils import run_bass_kernel_spmd

BF16 = ml_dtypes.bfloat16

B, S, D = 2, 1024, 4096
H, KVH, HD = 32, 8, 128
GROUPS = H // KVH
LAT, LH = 1024, 32
THETA = 10000.0
SCALE = 1.0 / float(np.sqrt(HD))

NCORES = 8
TP = 4                 # head-group shards
HL = H // TP           # 8 local q heads
KVL = KVH // TP        # 2 local kv heads
LHL = LH // TP         # 8 local latent heads

f32 = mybir.dt.float32
bf16 = mybir.dt.bfloat16

D_T = D // 128         # 32 k-tiles over model dim
S_T = S // 128         # 8 token tiles of 128
IB = 2                 # token blocks of 512
NB = D // 512          # 8 output column blocks


def _build_program():
    nc = bacc.Bacc("TRN2", target_bir_lowering=False, debug=False)

    xt_d = nc.dram_tensor("xt", [128, D_T, S], bf16, kind="ExternalInput")
    wq_d = nc.dram_tensor("wq", [HL, 128, D_T, 128], bf16, kind="ExternalInput")
    wk_d = nc.dram_tensor("wk", [KVL, 128, D_T, 128], bf16, kind="ExternalInput")
    wv_d = nc.dram_tensor("wv", [128, D_T, KVL * HD], bf16, kind="ExternalInput")
    w2_d = nc.dram_tensor("w2", [LHL, 128, D_T, 128], bf16, kind="ExternalInput")
    wlv_d = nc.dram_tensor("wlv", [128, D_T, LHL * HD], bf16, kind="ExternalInput")
    wo_d = nc.dram_tensor("wo", [NB, 128, HL, 512], bf16, kind="ExternalInput")
    wlo_d = nc.dram_tensor("wlo", [NB, 128, LHL, 512], bf16, kind="ExternalInput")
    cos_d = nc.dram_tensor("cosT", [HD, S], f32, kind="ExternalInput")
    sin_d = nc.dram_tensor("sinTs", [HD, S], f32, kind="ExternalInput")
    mask_d = nc.dram_tensor("maskD", [128, 128], bf16, kind="ExternalInput")
    out_d = nc.dram_tensor("out", [S, D], f32, kind="ExternalOutput")

    out_ap = out_d.ap().rearrange("(tt p) d -> p tt d", p=128)

    with tile.TileContext(nc) as tc:
        with tc.tile_pool(name="const", bufs=1) as constp, \
             tc.tile_pool(name="acts", bufs=1) as acts:

            # persistent activations (bf16); q/k/lk tiles allocated at B1
            v_sb = acts.tile([128, S_T, KVL * HD], bf16, tag="v")
            lv_sb = acts.tile([128, S_T, LHL * HD], bf16, tag="lv")

            with tc.tile_pool(name="xt", bufs=1) as xtp:
                xt = xtp.tile([128, D_T, S], bf16, tag="xt")

                # preload zone: B1's first weight tile + rope tables live in
                # memory that is never recycled from B2's streaming pools, so
                # their DMAs run during B2 compute instead of after it.
                wpre = xtp.tile([128, D_T, 128], bf16, tag="wpre")
                cosT = xtp.tile([HD, S], f32, tag="cosT")
                sinTs = xtp.tile([HD, S], f32, tag="sinTs")

                # ---- phase B2: token-major projections v, lv ----
                # wlv streamed in four 256-col quarters (double-buffered);
                # first-needed DMA chunks issued first, kt-quarter grain.
                with tc.tile_pool(name="wv", bufs=1) as wvp, \
                     tc.tile_pool(name="wlvq", bufs=2) as wlvp, \
                     tc.tile_pool(name="ps_b2", bufs=4, space="PSUM") as psb2, \
                     tc.tile_pool(name="ps_b2v", bufs=2, space="PSUM") as psb2v:
                    wv_sb = wvp.tile([128, D_T, KVL * HD], bf16, tag="wv")
                    wlv_q = [wlvp.tile([128, D_T, 256], bf16, tag="wlvq",
                                       name=f"wlvq_{q}") for q in range(4)]

                    def dma_q(q, c):
                        nc.sync.dma_start(
                            wlv_q[q][:, bass.ts(c, D_T // 8), :],
                            wlv_d.ap()[:, bass.ts(c, D_T // 8),
                                       bass.ds(q * 256, 256)])

                    # interleaved fine-grained input DMAs, first-needed first
                    for c in range(8):
                        nc.sync.dma_start(
                            xt[:, bass.ts(c, D_T // 8), :],
                            xt_d.ap()[:, bass.ts(c, D_T // 8), :])
                        nc.sync.dma_start(
                            wv_sb[:, bass.ts(c, D_T // 8), :],
                            wv_d.ap()[:, bass.ts(c, D_T // 8), :])
                        dma_q(0, c)
                    for c in range(8):
                        dma_q(1, c)
                    nc.sync.dma_start(wpre[:], wq_d.ap()[0])
                    nc.sync.dma_start(cosT[:], cos_d.ap())
                    nc.sync.dma_start(sinTs[:], sin_d.ap())

                    for q in range(4):
                        if q < 2:
                            for c in range(8):
                                dma_q(q + 2, c)
                        for tt in range(S_T):
                            ps_lv = psb2.tile([128, 512], f32, tag="ps_lv",
                                              name=f"ps_lv_{q}_{tt}")
                            ps_v = None
                            if q == 0:
                                ps_v = psb2v.tile(
                                    [128, 512], f32, tag="ps_v",
                                    name=f"ps_v_{tt}")
                            for kt in range(D_T):
                                st = kt == 0
                                sp = kt == D_T - 1
                                lhs = xt[:, kt, bass.ts(tt, 128)]
                                nc.tensor.matmul(ps_lv[:, 0:256], lhs,
                                                 wlv_q[q][:, kt, :],
                                                 start=st, stop=sp)
                                if q == 0:
                                    nc.tensor.matmul(ps_v[:, 0:256], lhs,
                                                     wv_sb[:, kt, :],
                                                     start=st, stop=sp)
                            nc.any.tensor_copy(
                                lv_sb[:, tt, bass.ds(q * 256, 256)],
                                ps_lv[:, 0:256])
                            if q == 0:
                                nc.any.tensor_copy(v_sb[:, tt, :],
                                                   ps_v[:, 0:256])

                # ---- phase B1: feature-major projections q, k, lk (+rope) --
                qT = acts.tile([128, HL, S], bf16, tag="qT")
                kT = acts.tile([128, KVL, S], bf16, tag="kT")
                lkT = acts.tile([128, LHL, S], bf16, tag="lkT")
                with tc.tile_pool(name="wstr", bufs=3) as wstr, \
                     tc.tile_pool(name="rope", bufs=4) as ropep, \
                     tc.tile_pool(name="ps_b1", bufs=4, space="PSUM") as psb1:
                    _uidb = [0]

                    def uname_b1(pfx):
                        _uidb[0] += 1
                        return f"{pfx}_{_uidb[0]}"

                    def rope_to(dst, ps, ib):
                        sl = bass.ts(ib, 512)
                        rt = ropep.tile([128, 512], f32, tag="rt", name=uname_b1("rt"))
                        qc = ropep.tile([128, 512], f32, tag="qc", name=uname_b1("qc"))
                        nc.vector.tensor_tensor(
                            rt[0:64, :], ps[64:128, :], sinTs[0:64, sl],
                            mybir.AluOpType.mult)
                        nc.vector.tensor_tensor(
                            rt[64:128, :], ps[0:64, :], sinTs[64:128, sl],
                            mybir.AluOpType.mult)
                        nc.vector.tensor_tensor(
                            qc[:], ps[:], cosT[:, sl], mybir.AluOpType.mult)
                        nc.vector.tensor_add(dst, qc[:], rt[:])

                    def proj_fm(w_dram, n_tiles, dst, pre=None):
                        for nt in range(n_tiles):
                            if nt == 0 and pre is not None:
                                wt = pre
                            else:
                                wt = wstr.tile([128, D_T, 128], bf16,
                                               tag="w_fm", name=uname_b1("w_fm"))
                                nc.sync.dma_start(wt[:], w_dram.ap()[nt])
                            ps = [psb1.tile([128, 512], f32, tag="ps_b1",
                                            name=f"ps_b1_{nt}_{ib}")
                                  for ib in range(IB)]
                            for kt in range(D_T):
                                for ib in range(IB):
                                    nc.tensor.matmul(
                                        ps[ib][:], wt[:, kt, :],
                                        xt[:, kt, bass.ts(ib, 512)],
                                        start=(kt == 0), stop=(kt == D_T - 1))
                            for ib in range(IB):
                                rope_to(dst[:, nt, bass.ts(ib, 512)],
                                        ps[ib][:], ib)

                    proj_fm(wq_d, HL, qT, pre=wpre)
                    proj_fm(wk_d, KVL, kT)
                    proj_fm(w2_d, LHL, lkT)

            # ---- phase C+D: attention with output-projection weave ----
            with tc.tile_pool(name="attnlat", bufs=1) as alp:
                attnT = alp.tile([128, HL, S], bf16, tag="attnT")
                latT = alp.tile([128, LHL, S], bf16, tag="latT")

                with tc.tile_pool(name="pp", bufs=16) as pp, \
                     tc.tile_pool(name="dn", bufs=3) as dn, \
                     tc.tile_pool(name="wop", bufs=4) as wop, \
                     tc.tile_pool(name="ost", bufs=6) as ost, \
                     tc.tile_pool(name="const2", bufs=1) as constp2:
                    maskD = constp2.tile([128, 128], bf16, tag="maskD")
                    ones = constp2.tile([128, 128], bf16, tag="ones")
                    nc.sync.dma_start(maskD[:], mask_d.ap())
                    nc.vector.memset(ones[:], 1.0)
                    wo_seq = [0]
                    _uid = [0]

                    def uname(pfx):
                        _uid[0] += 1
                        return f"{pfx}_{_uid[0]}"

                    def vh_parts(vh):
                        if vh < HL:
                            h = vh
                            return (h, kT[:, h // GROUPS, :], attnT,
                                    lambda jb: v_sb[:, jb,
                                                    bass.ts(h // GROUPS, HD)])
                        h = vh - HL
                        return (h, lkT[:, h, :], latT,
                                lambda jb: lv_sb[:, jb, bass.ts(h, HD)])

                    def wo_dma(nb):
                        sq = wo_seq[0]
                        wo_seq[0] += 1
                        w = wop.tile([128, HL, 512], bf16, tag="wo",
                                     name=f"wo_{sq}")
                        wl = wop.tile([128, LHL, 512], bf16, tag="wlo",
                                      name=f"wlo_{sq}")
                        nc.sync.dma_start(w[:], wo_d.ap()[nb])
                        nc.sync.dma_start(wl[:], wlo_d.ap()[nb])
                        return w, wl

                    def d_chunk(psf_pool, wpair, nb, tt):
                        w, wl = wpair
                        psf = psf_pool.tile([128, 512], f32, tag="psf", name=uname("psf"))
                        for h in range(HL):
                            nc.tensor.matmul(
                                psf[:], attnT[:, h, bass.ts(tt, 128)],
                                w[:, h, :], start=(h == 0), stop=False)
                        for h in range(LHL):
                            nc.tensor.matmul(
                                psf[:], latT[:, h, bass.ts(tt, 128)],
                                wl[:, h, :], start=False, stop=(h == LHL - 1))
                        ot = ost.tile([128, 512], f32, tag="ot", name=uname("ot"))
                        nc.vector.tensor_copy(ot[:], psf[:])
                        nc.sync.dma_start(
                            out_ap[:, tt, bass.ds(nb * 512, 512)], ot[:])

                    with tc.tile_pool(name="ps_s", bufs=3, space="PSUM") as pss_, \
                         tc.tile_pool(name="ps_o", bufs=2, space="PSUM") as pso_, \
                         tc.tile_pool(name="ps_d", bufs=1, space="PSUM") as psd_, \
                         tc.tile_pool(name="ps_f", bufs=2, space="PSUM") as psf_:

                        def off_of(jb, ib):
                            return max(jb - 4 * ib, 0) * 128

                        def emit_scores(u, half):
                            # scores + exp + mask for jbs of one half
                            vh, ib, pts = u
                            h, ksrc, _, _ = vh_parts(vh)
                            njb = 4 * (ib + 1)
                            lo = 0 if half == 0 else njb // 2
                            hi = njb // 2 if half == 0 else njb
                            for jb in range(lo, hi):
                                off = off_of(jb, ib)
                                ps_s = pss_.tile([128, 512], f32, tag="ps_s", name=uname("ps_s"))
                                nc.tensor.matmul(
                                    ps_s[:, off:512],
                                    ksrc[:, bass.ts(jb, 128)],
                                    qT[:, h, bass.ds(ib * 512 + off,
                                                     512 - off)],
                                    start=True, stop=True)
                                pt = pp.tile([128, 512], bf16, tag="pt", name=uname("pt"))
                                nc.scalar.activation(
                                    pt[:, off:512], ps_s[:, off:512],
                                    mybir.ActivationFunctionType.Exp,
                                    scale=SCALE)
                                if jb >= 4 * ib:
                                    nc.gpsimd.tensor_mul(
                                        pt[:, off:off + 128],
                                        pt[:, off:off + 128], maskD[:])
                                pts.append(pt)

                        def emit_pv(u):
                            vh, ib, pts = u
                            _, _, _, vsl = vh_parts(vh)
                            njb = 4 * (ib + 1)
                            u.append(pso_.tile([128, 512], f32, tag="ps_o", name=uname("ps_o")))
                            ps_o = u[3]
                            for jb in range(njb):
                                off = off_of(jb, ib)
                                nc.tensor.matmul(
                                    ps_o[:, off:512], vsl(jb),
                                    pts[jb][:, off:512],
                                    start=(jb == 0), stop=(jb == njb - 1))

                        def emit_den_norm(u):
                            vh, ib, pts, ps_o = u
                            h, _, dst, _ = vh_parts(vh)
                            njb = 4 * (ib + 1)
                            ps_d = psd_.tile([128, 512], f32, tag="ps_d", name=uname("ps_d"))
                            for jb in range(njb):
                                off = off_of(jb, ib)
                                nc.tensor.matmul(
                                    ps_d[:, off:512], ones[:],
                                    pts[jb][:, off:512],
                                    start=(jb == 0), stop=(jb == njb - 1))
                            rec = dn.tile([128, 512], f32, tag="rec", name=uname("rec"))
                            nc.vector.reciprocal_approx_fast(rec[:], ps_d[:])
                            nc.vector.tensor_tensor(
                                dst[:, h, bass.ts(ib, 512)], ps_o[:], rec[:],
                                mybir.AluOpType.mult)

                        # --- pass ib=0: software-pipelined, one unit of lag
                        wpairs = {0: wo_dma(0), 1: wo_dma(1)}
                        prev = None
                        for vh in range(HL + LHL):
                            u = [vh, 0, []]
                            emit_scores(u, 0)
                            emit_scores(u, 1)
                            if prev is not None:
                                emit_pv(prev)
                                emit_den_norm(prev)
                            prev = u

                        # --- pass ib=1 with output-projection chunks woven
                        chunks = [(nb, tt) for nb in range(NB)
                                  for tt in range(S_T // 2)]
                        ci = 0
                        for vh in range(HL + LHL):
                            u = [vh, 1, []]
                            emit_scores(u, 0)
                            emit_pv(prev)
                            emit_scores(u, 1)
                            emit_den_norm(prev)
                            prev = u
                            for _ in range(2):
                                if ci < len(chunks):
                                    nb, tt = chunks[ci]
                                    for ahead in (1, 2):
                                        nba = nb + ahead
                                        if (nba < NB and nba not in wpairs
                                                and tt == 2 * ahead - 2):
                                            wpairs[nba] = wo_dma(nba)
                                    d_chunk(psf_, wpairs[nb], nb, tt)
                                    wpairs.pop(nb - 1, None)
                                    ci += 1
                        emit_pv(prev)
                        emit_den_norm(prev)

                    # --- output-projection tail: token tiles 4..7
                    with tc.tile_pool(name="ps_f2", bufs=4,
                                      space="PSUM") as psf2_:
                        tpair = {0: wo_dma(0), 1: wo_dma(1)}
                        for nb in range(NB):
                            for tt in range(S_T // 2, S_T):
                                if tt == S_T // 2 and nb + 2 < NB:
                                    tpair[nb + 2] = wo_dma(nb + 2)
                                d_chunk(psf2_, tpair[nb], nb, tt)
                            tpair.pop(nb, None)

    nc.compile()
    return nc


_NC = None


def _get_program():
    global _NC
    if _NC is None:
        _NC = _build_program()
    return _NC


def _rope_tables():
    inv_freq = 1.0 / (THETA ** (np.arange(0, HD, 2, dtype=np.float32) / HD))
    t = np.arange(S, dtype=np.float32)
    freqs = np.outer(t, inv_freq)                       # [S, 64]
    emb = np.concatenate([freqs, freqs], axis=-1)       # [S, HD]
    cosT = np.cos(emb).T.astype(np.float32).copy()      # [HD, S]
    sinT = np.sin(emb).T.astype(np.float32)
    sinTs = np.concatenate([-sinT[:HD // 2], sinT[HD // 2:]], 0).astype(
        np.float32).copy()
    return cosT, sinTs


def _mask_diag():
    # maskD[p, i] = 1.0 iff p <= i (upper-triangular incl. diagonal)
    p = np.arange(128)[:, None]
    i = np.arange(128)[None, :]
    return (p <= i).astype(BF16)


def _tile_w_fm(w, n_tiles, kt):
    # [K, n_tiles*128] -> [n_tiles, 128(p of K), kt, 128]
    K, N = w.shape
    assert K == kt * 128 and N == n_tiles * 128
    return np.ascontiguousarray(
        w.reshape(kt, 128, n_tiles, 128).transpose(2, 1, 0, 3)).astype(BF16)


def _tile_w_tm(w, kt):
    # [K, N] -> [128(p of K), kt, N]
    K, N = w.shape
    assert K == kt * 128
    return np.ascontiguousarray(
        w.reshape(kt, 128, N).transpose(1, 0, 2)).astype(BF16)


def _tile_w_out(w):
    # [1024, D] -> [8(nb), 128(p of rows), 8(h), 512]
    return np.ascontiguousarray(
        w.reshape(8, 128, D // 512, 512).transpose(2, 1, 0, 3)).astype(BF16)


def _make_in_maps(hidden_states, w_q, w_k, w_v, w_o, w_lq, w_lk, w_lv, w_lo):
    cosT, sinTs = _rope_tables()
    maskD = _mask_diag()
    w2 = np.asarray(w_lq, dtype=np.float32) @ np.asarray(w_lk,
                                                         dtype=np.float32)
    in_maps = []
    for c in range(NCORES):
        b, g = divmod(c, TP)
        x = np.asarray(hidden_states[b], dtype=np.float32)       # [S, D]
        xt = np.ascontiguousarray(
            x.T.reshape(D_T, 128, S).transpose(1, 0, 2)).astype(BF16)
        qs = slice(g * HL * HD, (g + 1) * HL * HD)
        kvs = slice(g * KVL * HD, (g + 1) * KVL * HD)
        ls = slice(g * LHL * HD, (g + 1) * LHL * HD)
        in_maps.append({
            "xt": xt,
            "wq": _tile_w_fm(np.asarray(w_q)[:, qs], HL, D_T),
            "wk": _tile_w_fm(np.asarray(w_k)[:, kvs], KVL, D_T),
            "wv": _tile_w_tm(np.asarray(w_v)[:, kvs], D_T),
            "w2": _tile_w_fm(w2[:, ls], LHL, D_T),
            "wlv": _tile_w_tm(np.asarray(w_lv)[:, ls], D_T),
            "wo": _tile_w_out(np.asarray(w_o)[qs, :]),
            "wlo": _tile_w_out(np.asarray(w_lo)[ls, :]),
            "cosT": cosT,
            "sinTs": sinTs,
            "maskD": maskD,
        })
    return in_maps


def kernel(hidden_states, w_q, w_k, w_v, w_o, w_lq, w_lk, w_lv, w_lo):
    nc = _get_program()
    in_maps = _make_in_maps(hidden_states, w_q, w_k, w_v, w_o,
                            w_lq, w_lk, w_lv, w_lo)
    res = run_bass_kernel_spmd(nc, in_maps, list(range(NCORES))).results

    out = np.zeros((B, S, D), dtype=np.float32)
    for c in range(NCORES):
        b = c // TP
        out[b] += res[c]["out"]
    return out


# revision 20
# speedup vs baseline: 1.0886x; 1.0239x over previous
"""Trainium2 Bass kernel for LlamaMultiheadLatentAttention.

Contract: kernel(**inputs) takes FULL fp32 inputs (as produced by
reference.setup_inputs) and returns the FULL fp32 output [2, 1024, 4096].

Sharding (8 cores, no collectives): core c handles batch b = c//4 and
head-group g = c%4 (8 query heads, 2 kv heads, 8 latent heads). q/k/v and
latent projections are column-sharded per head-group; o_proj/latent_o_proj
are row-sharded, so each core emits a partial output sum and the host adds
the 4 partials per batch (the "all-reduce" of the output happens at unshard
time on the host).

v2 layout/scheduling choices (on top of the v1 feature-major design):
  - w_lq @ w_lk premultiplied on the host: lk = x @ W2 directly, which
    shards the latent-q work that v1 duplicated across head-group cores.
  - causal trimming at 128-column granularity: score/exp/PV/denominator
    work below the diagonal j-block is skipped (PSUM accumulation into
    column subranges; the start=True matmul of each chain is full-width).
  - softmax denominator via a ones[128,128] stationary matmul, so the
    denominator lands broadcast across all 128 PSUM partitions and the
    normalization is a plain reciprocal+multiply (no partition broadcast).
  - attention is software-pipelined one unit (vh, ib) ahead: the PE does
    scores of unit u+1 between PV and denominator of unit u, giving the
    scalar engine (exp) a full unit of lead time.
  - the output projection is chopped into (col-block, token-tile) chunks;
    chunks for token tiles 0..3 are woven between attention units of the
    second token half as pure-PE filler, the rest run as a tail.
"""

import numpy as np
import ml_dtypes

import concourse.bass as bass
import concourse.mybir as mybir
import concourse.tile as tile
from concourse import bacc
from concourse.bass_utils import run_bass_kernel_spmd

BF16 = ml_dtypes.bfloat16

B, S, D = 2, 1024, 4096
H, KVH, HD = 32, 8, 128
GROUPS = H // KVH
LAT, LH = 1024, 32
THETA = 10000.0
SCALE = 1.0 / float(np.sqrt(HD))

NCORES = 8
TP = 4                 # head-group shards
HL = H // TP           # 8 local q heads
KVL = KVH // TP        # 2 local kv heads
LHL = LH // TP         # 8 local latent heads

f32 = mybir.dt.float32
bf16 = mybir.dt.bfloat16

D_T = D // 128         # 32 k-tiles over model dim
S_T = S // 128         # 8 token tiles of 128
IB = 2                 # token blocks of 512
NB = D // 512          # 8 output column blocks


def _build_program():
    nc = bacc.Bacc("TRN2", target_bir_lowering=False, debug=False)

    xt_d = nc.dram_tensor("xt", [128, D_T, S], bf16, kind="ExternalInput")
    wq_d = nc.dram_tensor("wq", [HL, 128, D_T, 128], bf16, kind="ExternalInput")
    wk_d = nc.dram_tensor("wk", [KVL, 128, D_T, 128], bf16, kind="ExternalInput")
    wv_d = nc.dram_tensor("wv", [128, D_T, KVL * HD], bf16, kind="ExternalInput")
    w2_d = nc.dram_tensor("w2", [LHL, 128, D_T, 128], bf16, kind="ExternalInput")
    wlv_d = nc.dram_tensor("wlv", [128, D_T, LHL * HD], bf16, kind="ExternalInput")
    wo_d = nc.dram_tensor("wo", [NB, 128, HL, 512], bf16, kind="ExternalInput")
    wlo_d = nc.dram_tensor("wlo", [NB, 128, LHL, 512], bf16, kind="ExternalInput")
    cos_d = nc.dram_tensor("cosT", [HD, S], f32, kind="ExternalInput")
    sin_d = nc.dram_tensor("sinTs", [HD, S], f32, kind="ExternalInput")
    mask_d = nc.dram_tensor("maskD", [128, 128], bf16, kind="ExternalInput")
    out_d = nc.dram_tensor("out", [S, D], f32, kind="ExternalOutput")

    out_ap = out_d.ap().rearrange("(tt p) d -> p tt d", p=128)

    with tile.TileContext(nc) as tc:
        with tc.tile_pool(name="const", bufs=1) as constp, \
             tc.tile_pool(name="acts", bufs=1) as acts:

            # persistent activations (bf16); q/k/lk tiles allocated at B1
            v_sb = acts.tile([128, S_T, KVL * HD], bf16, tag="v")
            lv_sb = acts.tile([128, S_T, LHL * HD], bf16, tag="lv")

            with tc.tile_pool(name="xt", bufs=1) as xtp:
                xt = xtp.tile([128, D_T, S], bf16, tag="xt")

                # preload zone: B1's first weight tile + rope tables live in
                # memory that is never recycled from B2's streaming pools, so
                # their DMAs run during B2 compute instead of after it.
                wpre = xtp.tile([128, D_T, 128], bf16, tag="wpre")
                cosT = xtp.tile([HD, S], f32, tag="cosT")
                sinTs = xtp.tile([HD, S], f32, tag="sinTs")

                # ---- phase B2: token-major projections v, lv ----
                # wlv streamed in four 256-col quarters (double-buffered);
                # first-needed DMA chunks issued first, kt-quarter grain.
                with tc.tile_pool(name="wv", bufs=1) as wvp, \
                     tc.tile_pool(name="wlvq", bufs=2) as wlvp, \
                     tc.tile_pool(name="ps_b2", bufs=4, space="PSUM") as psb2, \
                     tc.tile_pool(name="ps_b2v", bufs=2, space="PSUM") as psb2v:
                    wv_sb = wvp.tile([128, D_T, KVL * HD], bf16, tag="wv")
                    wlv_q = [wlvp.tile([128, D_T, 256], bf16, tag="wlvq",
                                       name=f"wlvq_{q}") for q in range(4)]

                    def dma_q(q, c):
                        nc.sync.dma_start(
                            wlv_q[q][:, bass.ts(c, D_T // 8), :],
                            wlv_d.ap()[:, bass.ts(c, D_T // 8),
                                       bass.ds(q * 256, 256)])

                    # interleaved fine-grained input DMAs, first-needed first
                    for c in range(8):
                        nc.sync.dma_start(
                            xt[:, bass.ts(c, D_T // 8), :],
                            xt_d.ap()[:, bass.ts(c, D_T // 8), :])
                        nc.sync.dma_start(
                            wv_sb[:, bass.ts(c, D_T // 8), :],
                            wv_d.ap()[:, bass.ts(c, D_T // 8), :])
                        dma_q(0, c)
                    for c in range(8):
                        dma_q(1, c)
                    nc.sync.dma_start(wpre[:], wq_d.ap()[0])
                    nc.sync.dma_start(cosT[:], cos_d.ap())
                    nc.sync.dma_start(sinTs[:], sin_d.ap())

                    for q in range(4):
                        if q < 2:
                            for c in range(8):
                                dma_q(q + 2, c)
                        for tt in range(S_T):
                            ps_lv = psb2.tile([128, 512], f32, tag="ps_lv",
                                              name=f"ps_lv_{q}_{tt}")
                            ps_v = None
                            if q == 0:
                                ps_v = psb2v.tile(
                                    [128, 512], f32, tag="ps_v",
                                    name=f"ps_v_{tt}")
                            for kt in range(D_T):
                                st = kt == 0
                                sp = kt == D_T - 1
                                lhs = xt[:, kt, bass.ts(tt, 128)]
                                nc.tensor.matmul(ps_lv[:, 0:256], lhs,
                                                 wlv_q[q][:, kt, :],
                                                 start=st, stop=sp)
                                if q == 0:
                                    nc.tensor.matmul(ps_v[:, 0:256], lhs,
                                                     wv_sb[:, kt, :],
                                                     start=st, stop=sp)
                            nc.any.tensor_copy(
                                lv_sb[:, tt, bass.ds(q * 256, 256)],
                                ps_lv[:, 0:256])
                            if q == 0:
                                nc.any.tensor_copy(v_sb[:, tt, :],
                                                   ps_v[:, 0:256])

                # ---- phase B1: feature-major projections q, k, lk (+rope) --
                qT = acts.tile([128, HL, S], bf16, tag="qT")
                kT = acts.tile([128, KVL, S], bf16, tag="kT")
                lkT = acts.tile([128, LHL, S], bf16, tag="lkT")
                with tc.tile_pool(name="wstr", bufs=3) as wstr, \
                     tc.tile_pool(name="rope", bufs=4) as ropep, \
                     tc.tile_pool(name="ps_b1", bufs=4, space="PSUM") as psb1:
                    _uidb = [0]

                    def uname_b1(pfx):
                        _uidb[0] += 1
                        return f"{pfx}_{_uidb[0]}"

                    def rope_to(dst, ps, ib):
                        sl = bass.ts(ib, 512)
                        rt = ropep.tile([128, 512], f32, tag="rt", name=uname_b1("rt"))
                        qc = ropep.tile([128, 512], f32, tag="qc", name=uname_b1("qc"))
                        nc.vector.tensor_tensor(
                            rt[0:64, :], ps[64:128, :], sinTs[0:64, sl],
                            mybir.AluOpType.mult)
                        nc.vector.tensor_tensor(
                            rt[64:128, :], ps[0:64, :], sinTs[64:128, sl],
                            mybir.AluOpType.mult)
                        nc.vector.tensor_tensor(
                            qc[:], ps[:], cosT[:, sl], mybir.AluOpType.mult)
                        nc.vector.tensor_add(dst, qc[:], rt[:])

                    def proj_fm(w_dram, n_tiles, dst, pre=None):
                        for nt in range(n_tiles):
                            if nt == 0 and pre is not None:
                                wt = pre
                            else:
                                wt = wstr.tile([128, D_T, 128], bf16,
                                               tag="w_fm", name=uname_b1("w_fm"))
                                nc.sync.dma_start(wt[:], w_dram.ap()[nt])
                            ps = [psb1.tile([128, 512], f32, tag="ps_b1",
                                            name=f"ps_b1_{nt}_{ib}")
                                  for ib in range(IB)]
                            for kt in range(D_T):
                                for ib in range(IB):
                                    nc.tensor.matmul(
                                        ps[ib][:], wt[:, kt, :],
                                        xt[:, kt, bass.ts(ib, 512)],
                                        start=(kt == 0), stop=(kt == D_T - 1))
                            for ib in range(IB):
                                rope_to(dst[:, nt, bass.ts(ib, 512)],
                                        ps[ib][:], ib)

                    proj_fm(wq_d, HL, qT, pre=wpre)
                    proj_fm(wk_d, KVL, kT)
                    proj_fm(w2_d, LHL, lkT)

            # ---- phase C+D: attention with output-projection weave ----
            with tc.tile_pool(name="attnlat", bufs=1) as alp:
                attnT = alp.tile([128, HL, S], bf16, tag="attnT")
                latT = alp.tile([128, LHL, S], bf16, tag="latT")

                with tc.tile_pool(name="pp", bufs=16) as pp, \
                     tc.tile_pool(name="dn", bufs=3) as dn, \
                     tc.tile_pool(name="wop", bufs=4) as wop, \
                     tc.tile_pool(name="ost", bufs=6) as ost, \
                     tc.tile_pool(name="const2", bufs=1) as constp2:
                    maskD = constp2.tile([128, 128], bf16, tag="maskD")
                    ones = constp2.tile([128, 128], bf16, tag="ones")
                    ones_f = constp2.tile([128, 128], f32, tag="ones_f")
                    ones32 = constp2.tile([128, 128], mybir.dt.float32r,
                                          tag="ones32")
                    nc.sync.dma_start(maskD[:], mask_d.ap())
                    nc.vector.memset(ones[:], 1.0)
                    nc.vector.memset(ones_f[:], 1.0)
                    nc.vector.tensor_copy(ones32[:], ones_f[:])
                    wo_seq = [0]
                    _uid = [0]

                    def uname(pfx):
                        _uid[0] += 1
                        return f"{pfx}_{_uid[0]}"

                    def vh_parts(vh):
                        if vh < HL:
                            h = vh
                            return (h, kT[:, h // GROUPS, :], attnT,
                                    lambda jb: v_sb[:, jb,
                                                    bass.ts(h // GROUPS, HD)])
                        h = vh - HL
                        return (h, lkT[:, h, :], latT,
                                lambda jb: lv_sb[:, jb, bass.ts(h, HD)])

                    def wo_dma(nb):
                        sq = wo_seq[0]
                        wo_seq[0] += 1
                        w = wop.tile([128, HL, 512], bf16, tag="wo",
                                     name=f"wo_{sq}")
                        wl = wop.tile([128, LHL, 512], bf16, tag="wlo",
                                      name=f"wlo_{sq}")
                        nc.sync.dma_start(w[:], wo_d.ap()[nb])
                        nc.sync.dma_start(wl[:], wlo_d.ap()[nb])
                        return w, wl

                    def d_chunk(psf_pool, wpair, nb, tt):
                        w, wl = wpair
                        psf = psf_pool.tile([128, 512], f32, tag="psf", name=uname("psf"))
                        for h in range(HL):
                            nc.tensor.matmul(
                                psf[:], attnT[:, h, bass.ts(tt, 128)],
                                w[:, h, :], start=(h == 0), stop=False)
                        for h in range(LHL):
                            nc.tensor.matmul(
                                psf[:], latT[:, h, bass.ts(tt, 128)],
                                wl[:, h, :], start=False, stop=(h == LHL - 1))
                        ot = ost.tile([128, 512], f32, tag="ot", name=uname("ot"))
                        nc.vector.tensor_copy(ot[:], psf[:])
                        nc.sync.dma_start(
                            out_ap[:, tt, bass.ds(nb * 512, 512)], ot[:])

                    with tc.tile_pool(name="ps_s", bufs=3, space="PSUM") as pss_, \
                         tc.tile_pool(name="ps_o", bufs=2, space="PSUM") as pso_, \
                         tc.tile_pool(name="ps_d", bufs=1, space="PSUM") as psd_, \
                         tc.tile_pool(name="ps_f", bufs=2, space="PSUM") as psf_:

                        def off_of(jb, ib):
                            return max(jb - 4 * ib, 0) * 128

                        def emit_scores(u, half):
                            # scores + exp + mask for jbs of one half
                            vh, ib, pts = u
                            h, ksrc, _, _ = vh_parts(vh)
                            njb = 4 * (ib + 1)
                            lo = 0 if half == 0 else njb // 2
                            hi = njb // 2 if half == 0 else njb
                            for jb in range(lo, hi):
                                off = off_of(jb, ib)
                                ps_s = pss_.tile([128, 512], f32, tag="ps_s", name=uname("ps_s"))
                                nc.tensor.matmul(
                                    ps_s[:, off:512],
                                    ksrc[:, bass.ts(jb, 128)],
                                    qT[:, h, bass.ds(ib * 512 + off,
                                                     512 - off)],
                                    start=True, stop=True)
                                pt = pp.tile([128, 512], bf16, tag="pt", name=uname("pt"))
                                nc.scalar.activation(
                                    pt[:, off:512], ps_s[:, off:512],
                                    mybir.ActivationFunctionType.Exp,
                                    scale=SCALE)
                                if jb >= 4 * ib:
                                    nc.gpsimd.tensor_mul(
                                        pt[:, off:off + 128],
                                        pt[:, off:off + 128], maskD[:])
                                pts.append(pt)

                        def emit_pv(u):
                            vh, ib, pts = u
                            _, _, _, vsl = vh_parts(vh)
                            njb = 4 * (ib + 1)
                            u.append(pso_.tile([128, 512], f32, tag="ps_o", name=uname("ps_o")))
                            ps_o = u[3]
                            for jb in range(njb):
                                off = off_of(jb, ib)
                                nc.tensor.matmul(
                                    ps_o[:, off:512], vsl(jb),
                                    pts[jb][:, off:512],
                                    start=(jb == 0), stop=(jb == njb - 1))

                        def emit_den_norm(u):
                            vh, ib, pts, ps_o = u
                            h, _, dst, _ = vh_parts(vh)
                            njb = 4 * (ib + 1)
                            ps_d = psd_.tile([128, 512], f32, tag="ps_d", name=uname("ps_d"))
                            if ib == 1:
                                # pre-sum the 8 P tiles on DVE (f32), then one
                                # f32r ones-matmul does the partition reduce
                                acc = dn.tile([128, 512],
                                              mybir.dt.float32r, tag="acc",
                                              name=uname("acc"))
                                nc.vector.tensor_add(acc[:], pts[0][:],
                                                     pts[1][:])
                                for jb in range(2, njb):
                                    off = off_of(jb, ib)
                                    nc.vector.tensor_add(
                                        acc[:, off:512], acc[:, off:512],
                                        pts[jb][:, off:512])
                                nc.tensor.matmul(
                                    ps_d[:], ones32[:], acc[:],
                                    start=True, stop=True)
                            else:
                                for jb in range(njb):
                                    off = off_of(jb, ib)
                                    nc.tensor.matmul(
                                        ps_d[:, off:512], ones[:],
                                        pts[jb][:, off:512],
                                        start=(jb == 0), stop=(jb == njb - 1))
                            rec = dn.tile([128, 512], f32, tag="rec", name=uname("rec"))
                            nc.vector.reciprocal_approx_fast(rec[:], ps_d[:])
                            nc.vector.tensor_tensor(
                                dst[:, h, bass.ts(ib, 512)], ps_o[:], rec[:],
                                mybir.AluOpType.mult)

                        # --- pass ib=0: software-pipelined, one unit of lag
                        wpairs = {0: wo_dma(0), 1: wo_dma(1)}
                        prev = None
                        for vh in range(HL + LHL):
                            u = [vh, 0, []]
                            emit_scores(u, 0)
                            emit_scores(u, 1)
                            if prev is not None:
                                emit_pv(prev)
                                emit_den_norm(prev)
                            prev = u

                        # --- pass ib=1 with output-projection chunks woven
                        chunks = [(nb, tt) for nb in range(NB)
                                  for tt in range(S_T // 2)]
                        ci = 0
                        for vh in range(HL + LHL):
                            u = [vh, 1, []]
                            emit_scores(u, 0)
                            emit_pv(prev)
                            emit_scores(u, 1)
                            emit_den_norm(prev)
                            prev = u
                            for _ in range(2):
                                if ci < len(chunks):
                                    nb, tt = chunks[ci]
                                    for ahead in (1, 2):
                                        nba = nb + ahead
                                        if (nba < NB and nba not in wpairs
                                                and tt == 2 * ahead - 2):
                                            wpairs[nba] = wo_dma(nba)
                                    d_chunk(psf_, wpairs[nb], nb, tt)
                                    wpairs.pop(nb - 1, None)
                                    ci += 1
                        emit_pv(prev)
                        emit_den_norm(prev)

                    # --- output-projection tail: token tiles 4..7
                    with tc.tile_pool(name="ps_f2", bufs=4,
                                      space="PSUM") as psf2_:
                        tpair = {0: wo_dma(0), 1: wo_dma(1)}
                        for nb in range(NB):
                            for tt in range(S_T // 2, S_T):
                                if tt == S_T // 2 and nb + 2 < NB:
                                    tpair[nb + 2] = wo_dma(nb + 2)
                                d_chunk(psf2_, tpair[nb], nb, tt)
                            tpair.pop(nb, None)

    nc.compile()
    return nc


_NC = None


def _get_program():
    global _NC
    if _NC is None:
        _NC = _build_program()
    return _NC


def _rope_tables():
    inv_freq = 1.0 / (THETA ** (np.arange(0, HD, 2, dtype=np.float32) / HD))
    t = np.arange(S, dtype=np.float32)
    freqs = np.outer(t, inv_freq)                       # [S, 64]
    emb = np.concatenate([freqs, freqs], axis=-1)       # [S, HD]
    cosT = np.cos(emb).T.astype(np.float32).copy()      # [HD, S]
    sinT = np.sin(emb).T.astype(np.float32)
    sinTs = np.concatenate([-sinT[:HD // 2], sinT[HD // 2:]], 0).astype(
        np.float32).copy()
    return cosT, sinTs


def _mask_diag():
    # maskD[p, i] = 1.0 iff p <= i (upper-triangular incl. diagonal)
    p = np.arange(128)[:, None]
    i = np.arange(128)[None, :]
    return (p <= i).astype(BF16)


def _tile_w_fm(w, n_tiles, kt):
    # [K, n_tiles*128] -> [n_tiles, 128(p of K), kt, 128]
    K, N = w.shape
    assert K == kt * 128 and N == n_tiles * 128
    return np.ascontiguousarray(
        w.reshape(kt, 128, n_tiles, 128).transpose(2, 1, 0, 3)).astype(BF16)


def _tile_w_tm(w, kt):
    # [K, N] -> [128(p of K), kt, N]
    K, N = w.shape
    assert K == kt * 128
    return np.ascontiguousarray(
        w.reshape(kt, 128, N).transpose(1, 0, 2)).astype(BF16)


def _tile_w_out(w):
    # [1024, D] -> [8(nb), 128(p of rows), 8(h), 512]
    return np.ascontiguousarray(
        w.reshape(8, 128, D // 512, 512).transpose(2, 1, 0, 3)).astype(BF16)


def _make_in_maps(hidden_states, w_q, w_k, w_v, w_o, w_lq, w_lk, w_lv, w_lo):
    cosT, sinTs = _rope_tables()
    maskD = _mask_diag()
    w2 = np.asarray(w_lq, dtype=np.float32) @ np.asarray(w_lk,
                                                         dtype=np.float32)
    in_maps = []
    for c in range(NCORES):
        b, g = divmod(c, TP)
        x = np.asarray(hidden_states[b], dtype=np.float32)       # [S, D]
        xt = np.ascontiguousarray(
            x.T.reshape(D_T, 128, S).transpose(1, 0, 2)).astype(BF16)
        qs = slice(g * HL * HD, (g + 1) * HL * HD)
        kvs = slice(g * KVL * HD, (g + 1) * KVL * HD)
        ls = slice(g * LHL * HD, (g + 1) * LHL * HD)
        in_maps.append({
            "xt": xt,
            "wq": _tile_w_fm(np.asarray(w_q)[:, qs], HL, D_T),
            "wk": _tile_w_fm(np.asarray(w_k)[:, kvs], KVL, D_T),
            "wv": _tile_w_tm(np.asarray(w_v)[:, kvs], D_T),
            "w2": _tile_w_fm(w2[:, ls], LHL, D_T),
            "wlv": _tile_w_tm(np.asarray(w_lv)[:, ls], D_T),
            "wo": _tile_w_out(np.asarray(w_o)[qs, :]),
            "wlo": _tile_w_out(np.asarray(w_lo)[ls, :]),
            "cosT": cosT,
            "sinTs": sinTs,
            "maskD": maskD,
        })
    return in_maps


def kernel(hidden_states, w_q, w_k, w_v, w_o, w_lq, w_lk, w_lv, w_lo):
    nc = _get_program()
    in_maps = _make_in_maps(hidden_states, w_q, w_k, w_v, w_o,
                            w_lq, w_lk, w_lv, w_lo)
    res = run_bass_kernel_spmd(nc, in_maps, list(range(NCORES))).results

    out = np.zeros((B, S, D), dtype=np.float32)
    for c in range(NCORES):
        b = c // TP
        out[b] += res[c]["out"]
    return out


# revision 23
# speedup vs baseline: 1.1117x; 1.0212x over previous
"""Trainium2 Bass kernel for LlamaMultiheadLatentAttention.

Contract: kernel(**inputs) takes FULL fp32 inputs (as produced by
reference.setup_inputs) and returns the FULL fp32 output [2, 1024, 4096].

Sharding (8 cores, no collectives): core c handles batch b = c//4 and
head-group g = c%4 (8 query heads, 2 kv heads, 8 latent heads). q/k/v and
latent projections are column-sharded per head-group; o_proj/latent_o_proj
are row-sharded, so each core emits a partial output sum and the host adds
the 4 partials per batch (the "all-reduce" of the output happens at unshard
time on the host).

v2 layout/scheduling choices (on top of the v1 feature-major design):
  - w_lq @ w_lk premultiplied on the host: lk = x @ W2 directly, which
    shards the latent-q work that v1 duplicated across head-group cores.
  - causal trimming at 128-column granularity: score/exp/PV/denominator
    work below the diagonal j-block is skipped (PSUM accumulation into
    column subranges; the start=True matmul of each chain is full-width).
  - softmax denominator via a ones[128,128] stationary matmul, so the
    denominator lands broadcast across all 128 PSUM partitions and the
    normalization is a plain reciprocal+multiply (no partition broadcast).
  - attention is software-pipelined one unit (vh, ib) ahead: the PE does
    scores of unit u+1 between PV and denominator of unit u, giving the
    scalar engine (exp) a full unit of lead time.
  - the output projection is chopped into (col-block, token-tile) chunks;
    chunks for token tiles 0..3 are woven between attention units of the
    second token half as pure-PE filler, the rest run as a tail.
"""

import numpy as np
import ml_dtypes

import concourse.bass as bass
import concourse.mybir as mybir
import concourse.tile as tile
from concourse import bacc
from concourse.bass_utils import run_bass_kernel_spmd

BF16 = ml_dtypes.bfloat16

B, S, D = 2, 1024, 4096
H, KVH, HD = 32, 8, 128
GROUPS = H // KVH
LAT, LH = 1024, 32
THETA = 10000.0
SCALE = 1.0 / float(np.sqrt(HD))

NCORES = 8
TP = 4                 # head-group shards
HL = H // TP           # 8 local q heads
KVL = KVH // TP        # 2 local kv heads
LHL = LH // TP         # 8 local latent heads

f32 = mybir.dt.float32
bf16 = mybir.dt.bfloat16

D_T = D // 128         # 32 k-tiles over model dim
S_T = S // 128         # 8 token tiles of 128
IB = 2                 # token blocks of 512
NB = D // 512          # 8 output column blocks


def _build_program():
    nc = bacc.Bacc("TRN2", target_bir_lowering=False, debug=False)

    xt_d = nc.dram_tensor("xt", [128, D_T, S], bf16, kind="ExternalInput")
    wq_d = nc.dram_tensor("wq", [HL, 128, D_T, 128], bf16, kind="ExternalInput")
    wk_d = nc.dram_tensor("wk", [KVL, 128, D_T, 128], bf16, kind="ExternalInput")
    wvc_d = nc.dram_tensor("wvc", [128, D_T, 512], bf16, kind="ExternalInput")
    w2_d = nc.dram_tensor("w2", [LHL, 128, D_T, 128], bf16, kind="ExternalInput")
    wlvb_d = nc.dram_tensor("wlvb", [128, D_T, 512], bf16, kind="ExternalInput")
    wlvc_d = nc.dram_tensor("wlvc", [128, D_T, 256], bf16, kind="ExternalInput")
    wo_d = nc.dram_tensor("wo", [NB, 128, HL, 512], bf16, kind="ExternalInput")
    wlo_d = nc.dram_tensor("wlo", [NB, 128, LHL, 512], bf16, kind="ExternalInput")
    cos_d = nc.dram_tensor("cosT", [HD, S], f32, kind="ExternalInput")
    sin_d = nc.dram_tensor("sinTs", [HD, S], f32, kind="ExternalInput")
    mask_d = nc.dram_tensor("maskD", [128, 128], bf16, kind="ExternalInput")
    out_d = nc.dram_tensor("out", [S, D], f32, kind="ExternalOutput")

    out_ap = out_d.ap().rearrange("(tt p) d -> p tt d", p=128)

    with tile.TileContext(nc) as tc:
        with tc.tile_pool(name="const", bufs=1) as constp, \
             tc.tile_pool(name="acts", bufs=1) as acts:

            # persistent activations (bf16); q/k/lk tiles allocated at B1
            v_sb = acts.tile([128, S_T, KVL * HD], bf16, tag="v")
            lv_sb = acts.tile([128, S_T, LHL * HD], bf16, tag="lv")

            with tc.tile_pool(name="xt", bufs=1) as xtp:
                xt = xtp.tile([128, D_T, S], bf16, tag="xt")

                # preload zone: B1's first weight tile + rope tables live in
                # memory that is never recycled from B2's streaming pools, so
                # their DMAs run during B2 compute instead of after it.
                wpre = xtp.tile([128, D_T, 128], bf16, tag="wpre")
                cosT = xtp.tile([HD, S], f32, tag="cosT")
                sinTs = xtp.tile([HD, S], f32, tag="sinTs")

                # ---- phase B2: token-major projections v, lv ----
                # three column passes (A = [wlv 0:256 | wv], B = wlv 256:768,
                # C = wlv 768:1024), kt-block-major so the PE consumes input
                # chunks in DMA arrival order; one PSUM bank per token tile.
                with tc.tile_pool(name="wb2", bufs=2) as wb2p, \
                     tc.tile_pool(name="ps_b2", bufs=8, space="PSUM") as psb2:
                    wA = wb2p.tile([128, D_T, 512], bf16, tag="w512",
                                   name="wA")
                    wB = wb2p.tile([128, D_T, 512], bf16, tag="w512",
                                   name="wB")
                    wC = wb2p.tile([128, D_T, 512], bf16, tag="w512",
                                   name="wC")
                    for c in range(8):
                        nc.sync.dma_start(
                            xt[:, bass.ts(c, D_T // 8), :],
                            xt_d.ap()[:, bass.ts(c, D_T // 8), :])
                        nc.sync.dma_start(
                            wA[:, bass.ts(c, D_T // 8), :],
                            wvc_d.ap()[:, bass.ts(c, D_T // 8), :])
                    for c in range(8):
                        nc.sync.dma_start(
                            wB[:, bass.ts(c, D_T // 8), :],
                            wlvb_d.ap()[:, bass.ts(c, D_T // 8), :])
                    for c in range(8):
                        nc.sync.dma_start(
                            wC[:, bass.ts(c, D_T // 8), 0:256],
                            wlvc_d.ap()[:, bass.ts(c, D_T // 8), :])
                    nc.sync.dma_start(wpre[:], wq_d.ap()[0])
                    nc.sync.dma_start(cosT[:], cos_d.ap())
                    nc.sync.dma_start(sinTs[:], sin_d.ap())

                    for p, (wt, ncols) in enumerate(
                            ((wA, 512), (wB, 512), (wC, 256))):
                        pss_b2 = [psb2.tile([128, 512], f32, tag="ps_b2",
                                            name=f"psb2_{p}_{tt}")
                                  for tt in range(S_T)]
                        for kb in range(8):
                            for tt in range(S_T):
                                for kt in range(kb * 4, kb * 4 + 4):
                                    nc.tensor.matmul(
                                        pss_b2[tt][:, 0:ncols],
                                        xt[:, kt, bass.ts(tt, 128)],
                                        wt[:, kt, 0:ncols],
                                        start=(kt == 0), stop=(kt == D_T - 1))
                                if kb == 7:
                                    ps = pss_b2[tt]
                                    if p == 0:
                                        nc.any.tensor_copy(
                                            lv_sb[:, tt, 0:256], ps[:, 0:256])
                                        nc.any.tensor_copy(
                                            v_sb[:, tt, :], ps[:, 256:512])
                                    elif p == 1:
                                        nc.any.tensor_copy(
                                            lv_sb[:, tt, 256:768], ps[:])
                                    else:
                                        nc.any.tensor_copy(
                                            lv_sb[:, tt, 768:1024],
                                            ps[:, 0:256])

                # ---- phase B1: feature-major projections q, k, lk (+rope) --
                qT = acts.tile([128, HL, S], bf16, tag="qT")
                kT = acts.tile([128, KVL, S], bf16, tag="kT")
                lkT = acts.tile([128, LHL, S], bf16, tag="lkT")
                with tc.tile_pool(name="wstr", bufs=3) as wstr, \
                     tc.tile_pool(name="rope", bufs=4) as ropep, \
                     tc.tile_pool(name="ps_b1", bufs=4, space="PSUM") as psb1:
                    _uidb = [0]

                    def uname_b1(pfx):
                        _uidb[0] += 1
                        return f"{pfx}_{_uidb[0]}"

                    def rope_to(dst, ps, ib):
                        sl = bass.ts(ib, 512)
                        rt = ropep.tile([128, 512], f32, tag="rt", name=uname_b1("rt"))
                        qc = ropep.tile([128, 512], f32, tag="qc", name=uname_b1("qc"))
                        nc.vector.tensor_tensor(
                            rt[0:64, :], ps[64:128, :], sinTs[0:64, sl],
                            mybir.AluOpType.mult)
                        nc.vector.tensor_tensor(
                            rt[64:128, :], ps[0:64, :], sinTs[64:128, sl],
                            mybir.AluOpType.mult)
                        nc.vector.tensor_tensor(
                            qc[:], ps[:], cosT[:, sl], mybir.AluOpType.mult)
                        nc.vector.tensor_add(dst, qc[:], rt[:])

                    def proj_fm(w_dram, n_tiles, dst, pre=None):
                        for nt in range(n_tiles):
                            if nt == 0 and pre is not None:
                                wt = pre
                            else:
                                wt = wstr.tile([128, D_T, 128], bf16,
                                               tag="w_fm", name=uname_b1("w_fm"))
                                nc.sync.dma_start(wt[:], w_dram.ap()[nt])
                            ps = [psb1.tile([128, 512], f32, tag="ps_b1",
                                            name=f"ps_b1_{nt}_{ib}")
                                  for ib in range(IB)]
                            for kt in range(D_T):
                                for ib in range(IB):
                                    nc.tensor.matmul(
                                        ps[ib][:], wt[:, kt, :],
                                        xt[:, kt, bass.ts(ib, 512)],
                                        start=(kt == 0), stop=(kt == D_T - 1))
                            for ib in range(IB):
                                rope_to(dst[:, nt, bass.ts(ib, 512)],
                                        ps[ib][:], ib)

                    proj_fm(wq_d, HL, qT, pre=wpre)
                    proj_fm(wk_d, KVL, kT)
                    proj_fm(w2_d, LHL, lkT)

            # ---- phase C+D: attention with output-projection weave ----
            with tc.tile_pool(name="attnlat", bufs=1) as alp:
                attnT = alp.tile([128, HL, S], bf16, tag="attnT")
                latT = alp.tile([128, LHL, S], bf16, tag="latT")

                with tc.tile_pool(name="pp", bufs=16) as pp, \
                     tc.tile_pool(name="dn", bufs=3) as dn, \
                     tc.tile_pool(name="wop", bufs=4) as wop, \
                     tc.tile_pool(name="ost", bufs=6) as ost, \
                     tc.tile_pool(name="const2", bufs=1) as constp2:
                    maskD = constp2.tile([128, 128], bf16, tag="maskD")
                    ones = constp2.tile([128, 128], bf16, tag="ones")
                    ones_f = constp2.tile([128, 128], f32, tag="ones_f")
                    ones32 = constp2.tile([128, 128], mybir.dt.float32r,
                                          tag="ones32")
                    nc.sync.dma_start(maskD[:], mask_d.ap())
                    nc.vector.memset(ones[:], 1.0)
                    nc.vector.memset(ones_f[:], 1.0)
                    nc.vector.tensor_copy(ones32[:], ones_f[:])
                    wo_seq = [0]
                    _uid = [0]

                    def uname(pfx):
                        _uid[0] += 1
                        return f"{pfx}_{_uid[0]}"

                    def vh_parts(vh):
                        if vh < HL:
                            h = vh
                            return (h, kT[:, h // GROUPS, :], attnT,
                                    lambda jb: v_sb[:, jb,
                                                    bass.ts(h // GROUPS, HD)])
                        h = vh - HL
                        return (h, lkT[:, h, :], latT,
                                lambda jb: lv_sb[:, jb, bass.ts(h, HD)])

                    def wo_dma(nb):
                        sq = wo_seq[0]
                        wo_seq[0] += 1
                        w = wop.tile([128, HL, 512], bf16, tag="wo",
                                     name=f"wo_{sq}")
                        wl = wop.tile([128, LHL, 512], bf16, tag="wlo",
                                      name=f"wlo_{sq}")
                        nc.sync.dma_start(w[:], wo_d.ap()[nb])
                        nc.sync.dma_start(wl[:], wlo_d.ap()[nb])
                        return w, wl

                    def d_chunk(psf_pool, wpair, nb, tt):
                        w, wl = wpair
                        psf = psf_pool.tile([128, 512], f32, tag="psf", name=uname("psf"))
                        for h in range(HL):
                            nc.tensor.matmul(
                                psf[:], attnT[:, h, bass.ts(tt, 128)],
                                w[:, h, :], start=(h == 0), stop=False)
                        for h in range(LHL):
                            nc.tensor.matmul(
                                psf[:], latT[:, h, bass.ts(tt, 128)],
                                wl[:, h, :], start=False, stop=(h == LHL - 1))
                        ot = ost.tile([128, 512], f32, tag="ot", name=uname("ot"))
                        nc.vector.tensor_copy(ot[:], psf[:])
                        nc.sync.dma_start(
                            out_ap[:, tt, bass.ds(nb * 512, 512)], ot[:])

                    with tc.tile_pool(name="ps_f", bufs=2, space="PSUM") as psf_, \
                         tc.tile_pool(name="ps_o", bufs=2, space="PSUM") as pso_, \
                         tc.tile_pool(name="ps_d", bufs=1, space="PSUM") as psd_, \
                         tc.tile_pool(name="ps_s", bufs=3, space="PSUM") as pss_:

                        def off_of(jb, ib):
                            return max(jb - 4 * ib, 0) * 128

                        def emit_scores(u, half):
                            # scores + exp + mask for jbs of one half
                            vh, ib, pts = u
                            h, ksrc, _, _ = vh_parts(vh)
                            njb = 4 * (ib + 1)
                            lo = 0 if half == 0 else njb // 2
                            hi = njb // 2 if half == 0 else njb
                            for jb in range(lo, hi):
                                off = off_of(jb, ib)
                                ps_s = pss_.tile([128, 512], f32, tag="ps_s", name=uname("ps_s"))
                                nc.tensor.matmul(
                                    ps_s[:, off:512],
                                    ksrc[:, bass.ts(jb, 128)],
                                    qT[:, h, bass.ds(ib * 512 + off,
                                                     512 - off)],
                                    start=True, stop=True)
                                pt = pp.tile([128, 512], bf16, tag="pt", name=uname("pt"))
                                nc.scalar.activation(
                                    pt[:, off:512], ps_s[:, off:512],
                                    mybir.ActivationFunctionType.Exp,
                                    scale=SCALE)
                                if jb >= 4 * ib:
                                    nc.gpsimd.tensor_mul(
                                        pt[:, off:off + 128],
                                        pt[:, off:off + 128], maskD[:])
                                pts.append(pt)

                        def emit_pv(u):
                            vh, ib, pts = u
                            _, _, _, vsl = vh_parts(vh)
                            njb = 4 * (ib + 1)
                            u.append(pso_.tile([128, 512], f32, tag="ps_o", name=uname("ps_o")))
                            ps_o = u[3]
                            for jb in range(njb):
                                off = off_of(jb, ib)
                                nc.tensor.matmul(
                                    ps_o[:, off:512], vsl(jb),
                                    pts[jb][:, off:512],
                                    start=(jb == 0), stop=(jb == njb - 1))

                        def emit_den_norm(u):
                            vh, ib, pts, ps_o = u
                            h, _, dst, _ = vh_parts(vh)
                            njb = 4 * (ib + 1)
                            ps_d = psd_.tile([128, 512], f32, tag="ps_d", name=uname("ps_d"))
                            if ib == 1:
                                # pre-sum the 8 P tiles on DVE (f32), then one
                                # f32r ones-matmul does the partition reduce
                                acc = dn.tile([128, 512],
                                              mybir.dt.float32r, tag="acc",
                                              name=uname("acc"))
                                nc.vector.tensor_add(acc[:], pts[0][:],
                                                     pts[1][:])
                                for jb in range(2, njb):
                                    off = off_of(jb, ib)
                                    nc.vector.tensor_add(
                                        acc[:, off:512], acc[:, off:512],
                                        pts[jb][:, off:512])
                                nc.tensor.matmul(
                                    ps_d[:], ones32[:], acc[:],
                                    start=True, stop=True)
                            else:
                                for jb in range(njb):
                                    off = off_of(jb, ib)
                                    nc.tensor.matmul(
                                        ps_d[:, off:512], ones[:],
                                        pts[jb][:, off:512],
                                        start=(jb == 0), stop=(jb == njb - 1))
                            rec = dn.tile([128, 512], f32, tag="rec", name=uname("rec"))
                            nc.vector.reciprocal_approx_fast(rec[:], ps_d[:])
                            nc.vector.tensor_tensor(
                                dst[:, h, bass.ts(ib, 512)], ps_o[:], rec[:],
                                mybir.AluOpType.mult)

                        # --- pass ib=0: software-pipelined, one unit of lag
                        wpairs = {0: wo_dma(0), 1: wo_dma(1)}
                        prev = None
                        for vh in range(HL + LHL):
                            u = [vh, 0, []]
                            emit_scores(u, 0)
                            emit_scores(u, 1)
                            if prev is not None:
                                emit_pv(prev)
                                emit_den_norm(prev)
                            prev = u

                        # --- pass ib=1 with output-projection chunks woven
                        chunks = [(nb, tt) for nb in range(NB)
                                  for tt in range(S_T // 2)]
                        ci = 0
                        for vh in range(HL + LHL):
                            u = [vh, 1, []]
                            emit_scores(u, 0)
                            emit_pv(prev)
                            emit_scores(u, 1)
                            emit_den_norm(prev)
                            prev = u
                            for _ in range(2):
                                if ci < len(chunks):
                                    nb, tt = chunks[ci]
                                    for ahead in (1, 2):
                                        nba = nb + ahead
                                        if (nba < NB and nba not in wpairs
                                                and tt == 2 * ahead - 2):
                                            wpairs[nba] = wo_dma(nba)
                                    d_chunk(psf_, wpairs[nb], nb, tt)
                                    wpairs.pop(nb - 1, None)
                                    ci += 1
                        emit_pv(prev)
                        emit_den_norm(prev)

                    # --- output-projection tail: token tiles 4..7
                    with tc.tile_pool(name="ps_f2", bufs=4,
                                      space="PSUM") as psf2_:
                        tpair = {0: wo_dma(0), 1: wo_dma(1)}
                        for nb in range(NB):
                            for tt in range(S_T // 2, S_T):
                                if tt == S_T // 2 and nb + 2 < NB:
                                    tpair[nb + 2] = wo_dma(nb + 2)
                                d_chunk(psf2_, tpair[nb], nb, tt)
                            tpair.pop(nb, None)

    nc.compile()
    return nc


_NC = None


def _get_program():
    global _NC
    if _NC is None:
        _NC = _build_program()
    return _NC


def _rope_tables():
    inv_freq = 1.0 / (THETA ** (np.arange(0, HD, 2, dtype=np.float32) / HD))
    t = np.arange(S, dtype=np.float32)
    freqs = np.outer(t, inv_freq)                       # [S, 64]
    emb = np.concatenate([freqs, freqs], axis=-1)       # [S, HD]
    cosT = np.cos(emb).T.astype(np.float32).copy()      # [HD, S]
    sinT = np.sin(emb).T.astype(np.float32)
    sinTs = np.concatenate([-sinT[:HD // 2], sinT[HD // 2:]], 0).astype(
        np.float32).copy()
    return cosT, sinTs


def _mask_diag():
    # maskD[p, i] = 1.0 iff p <= i (upper-triangular incl. diagonal)
    p = np.arange(128)[:, None]
    i = np.arange(128)[None, :]
    return (p <= i).astype(BF16)


def _tile_w_fm(w, n_tiles, kt):
    # [K, n_tiles*128] -> [n_tiles, 128(p of K), kt, 128]
    K, N = w.shape
    assert K == kt * 128 and N == n_tiles * 128
    return np.ascontiguousarray(
        w.reshape(kt, 128, n_tiles, 128).transpose(2, 1, 0, 3)).astype(BF16)


def _tile_w_tm(w, kt):
    # [K, N] -> [128(p of K), kt, N]
    K, N = w.shape
    assert K == kt * 128
    return np.ascontiguousarray(
        w.reshape(kt, 128, N).transpose(1, 0, 2)).astype(BF16)


def _tile_w_out(w):
    # [1024, D] -> [8(nb), 128(p of rows), 8(h), 512]
    return np.ascontiguousarray(
        w.reshape(8, 128, D // 512, 512).transpose(2, 1, 0, 3)).astype(BF16)


def _make_in_maps(hidden_states, w_q, w_k, w_v, w_o, w_lq, w_lk, w_lv, w_lo):
    cosT, sinTs = _rope_tables()
    maskD = _mask_diag()
    w2 = np.asarray(w_lq, dtype=np.float32) @ np.asarray(w_lk,
                                                         dtype=np.float32)
    in_maps = []
    for c in range(NCORES):
        b, g = divmod(c, TP)
        x = np.asarray(hidden_states[b], dtype=np.float32)       # [S, D]
        xt = np.ascontiguousarray(
            x.T.reshape(D_T, 128, S).transpose(1, 0, 2)).astype(BF16)
        qs = slice(g * HL * HD, (g + 1) * HL * HD)
        kvs = slice(g * KVL * HD, (g + 1) * KVL * HD)
        ls = slice(g * LHL * HD, (g + 1) * LHL * HD)
        wv_t = _tile_w_tm(np.asarray(w_v)[:, kvs], D_T)
        wlv_t = _tile_w_tm(np.asarray(w_lv)[:, ls], D_T)
        in_maps.append({
            "xt": xt,
            "wq": _tile_w_fm(np.asarray(w_q)[:, qs], HL, D_T),
            "wk": _tile_w_fm(np.asarray(w_k)[:, kvs], KVL, D_T),
            "wvc": np.ascontiguousarray(
                np.concatenate([wlv_t[:, :, 0:256], wv_t], axis=2)),
            "w2": _tile_w_fm(w2[:, ls], LHL, D_T),
            "wlvb": np.ascontiguousarray(wlv_t[:, :, 256:768]),
            "wlvc": np.ascontiguousarray(wlv_t[:, :, 768:1024]),
            "wo": _tile_w_out(np.asarray(w_o)[qs, :]),
            "wlo": _tile_w_out(np.asarray(w_lo)[ls, :]),
            "cosT": cosT,
            "sinTs": sinTs,
            "maskD": maskD,
        })
    return in_maps


def kernel(hidden_states, w_q, w_k, w_v, w_o, w_lq, w_lk, w_lv, w_lo):
    nc = _get_program()
    in_maps = _make_in_maps(hidden_states, w_q, w_k, w_v, w_o,
                            w_lq, w_lk, w_lv, w_lo)
    res = run_bass_kernel_spmd(nc, in_maps, list(range(NCORES))).results

    out = np.zeros((B, S, D), dtype=np.float32)
    for c in range(NCORES):
        b = c // TP
        out[b] += res[c]["out"]
    return out


# revision 24
# speedup vs baseline: 1.1123x; 1.0005x over previous
"""Trainium2 Bass kernel for LlamaMultiheadLatentAttention.

Contract: kernel(**inputs) takes FULL fp32 inputs (as produced by
reference.setup_inputs) and returns the FULL fp32 output [2, 1024, 4096].

Sharding (8 cores, no collectives): core c handles batch b = c//4 and
head-group g = c%4 (8 query heads, 2 kv heads, 8 latent heads). q/k/v and
latent projections are column-sharded per head-group; o_proj/latent_o_proj
are row-sharded, so each core emits a partial output sum and the host adds
the 4 partials per batch (the "all-reduce" of the output happens at unshard
time on the host).

v2 layout/scheduling choices (on top of the v1 feature-major design):
  - w_lq @ w_lk premultiplied on the host: lk = x @ W2 directly, which
    shards the latent-q work that v1 duplicated across head-group cores.
  - causal trimming at 128-column granularity: score/exp/PV/denominator
    work below the diagonal j-block is skipped (PSUM accumulation into
    column subranges; the start=True matmul of each chain is full-width).
  - softmax denominator via a ones[128,128] stationary matmul, so the
    denominator lands broadcast across all 128 PSUM partitions and the
    normalization is a plain reciprocal+multiply (no partition broadcast).
  - attention is software-pipelined one unit (vh, ib) ahead: the PE does
    scores of unit u+1 between PV and denominator of unit u, giving the
    scalar engine (exp) a full unit of lead time.
  - the output projection is chopped into (col-block, token-tile) chunks;
    chunks for token tiles 0..3 are woven between attention units of the
    second token half as pure-PE filler, the rest run as a tail.
"""

import numpy as np
import ml_dtypes

import concourse.bass as bass
import concourse.mybir as mybir
import concourse.tile as tile
from concourse import bacc
from concourse.bass_utils import run_bass_kernel_spmd

BF16 = ml_dtypes.bfloat16

B, S, D = 2, 1024, 4096
H, KVH, HD = 32, 8, 128
GROUPS = H // KVH
LAT, LH = 1024, 32
THETA = 10000.0
SCALE = 1.0 / float(np.sqrt(HD))

NCORES = 8
TP = 4                 # head-group shards
HL = H // TP           # 8 local q heads
KVL = KVH // TP        # 2 local kv heads
LHL = LH // TP         # 8 local latent heads

f32 = mybir.dt.float32
bf16 = mybir.dt.bfloat16

D_T = D // 128         # 32 k-tiles over model dim
S_T = S // 128         # 8 token tiles of 128
IB = 2                 # token blocks of 512
NB = D // 512          # 8 output column blocks


def _build_program():
    nc = bacc.Bacc("TRN2", target_bir_lowering=False, debug=False)

    xt_d = nc.dram_tensor("xt", [128, D_T, S], bf16, kind="ExternalInput")
    wq_d = nc.dram_tensor("wq", [HL, 128, D_T, 128], bf16, kind="ExternalInput")
    wk_d = nc.dram_tensor("wk", [KVL, 128, D_T, 128], bf16, kind="ExternalInput")
    wvc_d = nc.dram_tensor("wvc", [128, D_T, 512], bf16, kind="ExternalInput")
    w2_d = nc.dram_tensor("w2", [LHL, 128, D_T, 128], bf16, kind="ExternalInput")
    wlvb_d = nc.dram_tensor("wlvb", [128, D_T, 512], bf16, kind="ExternalInput")
    wlvc_d = nc.dram_tensor("wlvc", [128, D_T, 256], bf16, kind="ExternalInput")
    wo_d = nc.dram_tensor("wo", [NB, 128, HL, 512], bf16, kind="ExternalInput")
    wlo_d = nc.dram_tensor("wlo", [NB, 128, LHL, 512], bf16, kind="ExternalInput")
    cos_d = nc.dram_tensor("cosT", [HD, S], f32, kind="ExternalInput")
    sin_d = nc.dram_tensor("sinTs", [HD, S], f32, kind="ExternalInput")
    mask_d = nc.dram_tensor("maskD", [128, 128], bf16, kind="ExternalInput")
    out_d = nc.dram_tensor("out", [S, D], f32, kind="ExternalOutput")

    out_ap = out_d.ap().rearrange("(tt p) d -> p tt d", p=128)

    with tile.TileContext(nc) as tc:
        with tc.tile_pool(name="const", bufs=1) as constp, \
             tc.tile_pool(name="acts", bufs=1) as acts:

            # persistent activations (bf16); q/k/lk tiles allocated at B1
            v_sb = acts.tile([128, S_T, KVL * HD], bf16, tag="v")
            lv_sb = acts.tile([128, S_T, LHL * HD], bf16, tag="lv")
            maskD = acts.tile([128, 128], bf16, tag="maskD")
            ones = acts.tile([128, 128], bf16, tag="ones")
            ones_f = acts.tile([128, 128], f32, tag="ones_f")
            ones32 = acts.tile([128, 128], mybir.dt.float32r, tag="ones32")
            nc.sync.dma_start(maskD[:], mask_d.ap())
            nc.vector.memset(ones[:], 1.0)
            nc.vector.memset(ones_f[:], 1.0)
            nc.vector.tensor_copy(ones32[:], ones_f[:])

            with tc.tile_pool(name="xt", bufs=1) as xtp:
                xt = xtp.tile([128, D_T, S], bf16, tag="xt")

                # preload zone: B1's first weight tile + rope tables live in
                # memory that is never recycled from B2's streaming pools, so
                # their DMAs run during B2 compute instead of after it.
                wpre = xtp.tile([128, D_T, 128], bf16, tag="wpre")
                cosT = xtp.tile([HD, S], f32, tag="cosT")
                sinTs = xtp.tile([HD, S], f32, tag="sinTs")

                # ---- phase B2: token-major projections v, lv ----
                # three column passes (A = [wlv 0:256 | wv], B = wlv 256:768,
                # C = wlv 768:1024), kt-block-major so the PE consumes input
                # chunks in DMA arrival order; one PSUM bank per token tile.
                with tc.tile_pool(name="wb2", bufs=2) as wb2p, \
                     tc.tile_pool(name="ps_b2", bufs=8, space="PSUM") as psb2:
                    wA = wb2p.tile([128, D_T, 512], bf16, tag="w512",
                                   name="wA")
                    wB = wb2p.tile([128, D_T, 512], bf16, tag="w512",
                                   name="wB")
                    wC = wb2p.tile([128, D_T, 512], bf16, tag="w512",
                                   name="wC")
                    for c in range(16):
                        nc.sync.dma_start(
                            xt[:, bass.ts(c, D_T // 16), :],
                            xt_d.ap()[:, bass.ts(c, D_T // 16), :])
                        nc.sync.dma_start(
                            wA[:, bass.ts(c, D_T // 16), :],
                            wvc_d.ap()[:, bass.ts(c, D_T // 16), :])
                    for c in range(8):
                        nc.sync.dma_start(
                            wB[:, bass.ts(c, D_T // 8), :],
                            wlvb_d.ap()[:, bass.ts(c, D_T // 8), :])
                    for c in range(8):
                        nc.sync.dma_start(
                            wC[:, bass.ts(c, D_T // 8), 0:256],
                            wlvc_d.ap()[:, bass.ts(c, D_T // 8), :])
                    nc.sync.dma_start(wpre[:], wq_d.ap()[0])
                    nc.sync.dma_start(cosT[:], cos_d.ap())
                    nc.sync.dma_start(sinTs[:], sin_d.ap())

                    for p, (wt, ncols) in enumerate(
                            ((wA, 512), (wB, 512), (wC, 256))):
                        pss_b2 = [psb2.tile([128, 512], f32, tag="ps_b2",
                                            name=f"psb2_{p}_{tt}")
                                  for tt in range(S_T)]
                        for kb in range(8):
                            for tt in range(S_T):
                                for kt in range(kb * 4, kb * 4 + 4):
                                    nc.tensor.matmul(
                                        pss_b2[tt][:, 0:ncols],
                                        xt[:, kt, bass.ts(tt, 128)],
                                        wt[:, kt, 0:ncols],
                                        start=(kt == 0), stop=(kt == D_T - 1))
                                if kb == 7:
                                    ps = pss_b2[tt]
                                    if p == 0:
                                        nc.any.tensor_copy(
                                            lv_sb[:, tt, 0:256], ps[:, 0:256])
                                        nc.any.tensor_copy(
                                            v_sb[:, tt, :], ps[:, 256:512])
                                    elif p == 1:
                                        nc.any.tensor_copy(
                                            lv_sb[:, tt, 256:768], ps[:])
                                    else:
                                        nc.any.tensor_copy(
                                            lv_sb[:, tt, 768:1024],
                                            ps[:, 0:256])

                # ---- phase B1: feature-major projections q, k, lk (+rope) --
                qT = acts.tile([128, HL, S], bf16, tag="qT")
                kT = acts.tile([128, KVL, S], bf16, tag="kT")
                lkT = acts.tile([128, LHL, S], bf16, tag="lkT")
                with tc.tile_pool(name="wstr", bufs=3) as wstr, \
                     tc.tile_pool(name="rope", bufs=4) as ropep, \
                     tc.tile_pool(name="ps_b1", bufs=4, space="PSUM") as psb1:
                    _uidb = [0]

                    def uname_b1(pfx):
                        _uidb[0] += 1
                        return f"{pfx}_{_uidb[0]}"

                    def rope_to(dst, ps, ib):
                        sl = bass.ts(ib, 512)
                        rt = ropep.tile([128, 512], f32, tag="rt", name=uname_b1("rt"))
                        qc = ropep.tile([128, 512], f32, tag="qc", name=uname_b1("qc"))
                        nc.vector.tensor_tensor(
                            rt[0:64, :], ps[64:128, :], sinTs[0:64, sl],
                            mybir.AluOpType.mult)
                        nc.vector.tensor_tensor(
                            rt[64:128, :], ps[0:64, :], sinTs[64:128, sl],
                            mybir.AluOpType.mult)
                        nc.vector.tensor_tensor(
                            qc[:], ps[:], cosT[:, sl], mybir.AluOpType.mult)
                        nc.vector.tensor_add(dst, qc[:], rt[:])

                    def proj_fm(w_dram, n_tiles, dst, pre=None):
                        for nt in range(n_tiles):
                            if nt == 0 and pre is not None:
                                wt = pre
                            else:
                                wt = wstr.tile([128, D_T, 128], bf16,
                                               tag="w_fm", name=uname_b1("w_fm"))
                                nc.sync.dma_start(wt[:], w_dram.ap()[nt])
                            ps = [psb1.tile([128, 512], f32, tag="ps_b1",
                                            name=f"ps_b1_{nt}_{ib}")
                                  for ib in range(IB)]
                            for kt in range(D_T):
                                for ib in range(IB):
                                    nc.tensor.matmul(
                                        ps[ib][:], wt[:, kt, :],
                                        xt[:, kt, bass.ts(ib, 512)],
                                        start=(kt == 0), stop=(kt == D_T - 1))
                            for ib in range(IB):
                                rope_to(dst[:, nt, bass.ts(ib, 512)],
                                        ps[ib][:], ib)

                    proj_fm(wq_d, HL, qT, pre=wpre)
                    proj_fm(wk_d, KVL, kT)
                    proj_fm(w2_d, LHL, lkT)

            # ---- phase C+D: attention with output-projection weave ----
            with tc.tile_pool(name="attnlat", bufs=1) as alp:
                attnT = alp.tile([128, HL, S], bf16, tag="attnT")
                latT = alp.tile([128, LHL, S], bf16, tag="latT")

                with tc.tile_pool(name="pp", bufs=16) as pp, \
                     tc.tile_pool(name="dn", bufs=3) as dn, \
                     tc.tile_pool(name="wop", bufs=4) as wop, \
                     tc.tile_pool(name="ost", bufs=6) as ost:
                    wo_seq = [0]
                    _uid = [0]

                    def uname(pfx):
                        _uid[0] += 1
                        return f"{pfx}_{_uid[0]}"

                    def vh_parts(vh):
                        if vh < HL:
                            h = vh
                            return (h, kT[:, h // GROUPS, :], attnT,
                                    lambda jb: v_sb[:, jb,
                                                    bass.ts(h // GROUPS, HD)])
                        h = vh - HL
                        return (h, lkT[:, h, :], latT,
                                lambda jb: lv_sb[:, jb, bass.ts(h, HD)])

                    def wo_dma(nb):
                        sq = wo_seq[0]
                        wo_seq[0] += 1
                        w = wop.tile([128, HL, 512], bf16, tag="wo",
                                     name=f"wo_{sq}")
                        wl = wop.tile([128, LHL, 512], bf16, tag="wlo",
                                      name=f"wlo_{sq}")
                        nc.sync.dma_start(w[:], wo_d.ap()[nb])
                        nc.sync.dma_start(wl[:], wlo_d.ap()[nb])
                        return w, wl

                    def d_chunk(psf_pool, wpair, nb, tt):
                        w, wl = wpair
                        psf = psf_pool.tile([128, 512], f32, tag="psf", name=uname("psf"))
                        for h in range(HL):
                            nc.tensor.matmul(
                                psf[:], attnT[:, h, bass.ts(tt, 128)],
                                w[:, h, :], start=(h == 0), stop=False)
                        for h in range(LHL):
                            nc.tensor.matmul(
                                psf[:], latT[:, h, bass.ts(tt, 128)],
                                wl[:, h, :], start=False, stop=(h == LHL - 1))
                        ot = ost.tile([128, 512], f32, tag="ot", name=uname("ot"))
                        nc.vector.tensor_copy(ot[:], psf[:])
                        nc.sync.dma_start(
                            out_ap[:, tt, bass.ds(nb * 512, 512)], ot[:])

                    with tc.tile_pool(name="ps_f", bufs=2, space="PSUM") as psf_, \
                         tc.tile_pool(name="ps_o", bufs=2, space="PSUM") as pso_, \
                         tc.tile_pool(name="ps_d", bufs=1, space="PSUM") as psd_, \
                         tc.tile_pool(name="ps_s", bufs=3, space="PSUM") as pss_:

                        def off_of(jb, ib):
                            return max(jb - 4 * ib, 0) * 128

                        def emit_scores(u, half):
                            # scores + exp + mask for jbs of one half
                            vh, ib, pts = u
                            h, ksrc, _, _ = vh_parts(vh)
                            njb = 4 * (ib + 1)
                            lo = 0 if half == 0 else njb // 2
                            hi = njb // 2 if half == 0 else njb
                            for jb in range(lo, hi):
                                off = off_of(jb, ib)
                                ps_s = pss_.tile([128, 512], f32, tag="ps_s", name=uname("ps_s"))
                                nc.tensor.matmul(
                                    ps_s[:, off:512],
                                    ksrc[:, bass.ts(jb, 128)],
                                    qT[:, h, bass.ds(ib * 512 + off,
                                                     512 - off)],
                                    start=True, stop=True)
                                pt = pp.tile([128, 512], bf16, tag="pt", name=uname("pt"))
                                nc.scalar.activation(
                                    pt[:, off:512], ps_s[:, off:512],
                                    mybir.ActivationFunctionType.Exp,
                                    scale=SCALE)
                                if jb >= 4 * ib:
                                    nc.gpsimd.tensor_mul(
                                        pt[:, off:off + 128],
                                        pt[:, off:off + 128], maskD[:])
                                pts.append(pt)

                        def emit_pv(u):
                            vh, ib, pts = u
                            _, _, _, vsl = vh_parts(vh)
                            njb = 4 * (ib + 1)
                            u.append(pso_.tile([128, 512], f32, tag="ps_o", name=uname("ps_o")))
                            ps_o = u[3]
                            for jb in range(njb):
                                off = off_of(jb, ib)
                                nc.tensor.matmul(
                                    ps_o[:, off:512], vsl(jb),
                                    pts[jb][:, off:512],
                                    start=(jb == 0), stop=(jb == njb - 1))

                        def emit_den_norm(u):
                            vh, ib, pts, ps_o = u
                            h, _, dst, _ = vh_parts(vh)
                            njb = 4 * (ib + 1)
                            ps_d = psd_.tile([128, 512], f32, tag="ps_d", name=uname("ps_d"))
                            if ib == 1:
                                # pre-sum the 8 P tiles on DVE (f32), then one
                                # f32r ones-matmul does the partition reduce
                                acc = dn.tile([128, 512],
                                              mybir.dt.float32r, tag="acc",
                                              name=uname("acc"))
                                nc.vector.tensor_add(acc[:], pts[0][:],
                                                     pts[1][:])
                                for jb in range(2, njb):
                                    off = off_of(jb, ib)
                                    nc.vector.tensor_add(
                                        acc[:, off:512], acc[:, off:512],
                                        pts[jb][:, off:512])
                                nc.tensor.matmul(
                                    ps_d[:], ones32[:], acc[:],
                                    start=True, stop=True)
                            else:
                                for jb in range(njb):
                                    off = off_of(jb, ib)
                                    nc.tensor.matmul(
                                        ps_d[:, off:512], ones[:],
                                        pts[jb][:, off:512],
                                        start=(jb == 0), stop=(jb == njb - 1))
                            rec = dn.tile([128, 512], f32, tag="rec", name=uname("rec"))
                            nc.vector.reciprocal_approx_fast(rec[:], ps_d[:])
                            nc.vector.tensor_tensor(
                                dst[:, h, bass.ts(ib, 512)], ps_o[:], rec[:],
                                mybir.AluOpType.mult)

                        # --- pass ib=0: software-pipelined, one unit of lag
                        wpairs = {0: wo_dma(0), 1: wo_dma(1)}
                        prev = None
                        for vh in range(HL + LHL):
                            u = [vh, 0, []]
                            emit_scores(u, 0)
                            emit_scores(u, 1)
                            if prev is not None:
                                emit_pv(prev)
                                emit_den_norm(prev)
                            prev = u

                        # --- pass ib=1 with output-projection chunks woven
                        chunks = [(nb, tt) for nb in range(NB)
                                  for tt in range(S_T // 2)]
                        ci = 0
                        for vh in range(HL + LHL):
                            u = [vh, 1, []]
                            emit_scores(u, 0)
                            emit_pv(prev)
                            emit_scores(u, 1)
                            emit_den_norm(prev)
                            prev = u
                            for _ in range(2):
                                if ci < len(chunks):
                                    nb, tt = chunks[ci]
                                    for ahead in (1, 2):
                                        nba = nb + ahead
                                        if (nba < NB and nba not in wpairs
                                                and tt == 2 * ahead - 2):
                                            wpairs[nba] = wo_dma(nba)
                                    d_chunk(psf_, wpairs[nb], nb, tt)
                                    wpairs.pop(nb - 1, None)
                                    ci += 1
                        emit_pv(prev)
                        emit_den_norm(prev)

                    # --- output-projection tail: token tiles 4..7
                    with tc.tile_pool(name="ps_f2", bufs=4,
                                      space="PSUM") as psf2_:
                        tpair = {0: wo_dma(0), 1: wo_dma(1)}
                        for nb in range(NB):
                            for tt in range(S_T // 2, S_T):
                                if tt == S_T // 2 and nb + 2 < NB:
                                    tpair[nb + 2] = wo_dma(nb + 2)
                                d_chunk(psf2_, tpair[nb], nb, tt)
                            tpair.pop(nb, None)

    nc.compile()
    return nc


_NC = None


def _get_program():
    global _NC
    if _NC is None:
        _NC = _build_program()
    return _NC


def _rope_tables():
    inv_freq = 1.0 / (THETA ** (np.arange(0, HD, 2, dtype=np.float32) / HD))
    t = np.arange(S, dtype=np.float32)
    freqs = np.outer(t, inv_freq)                       # [S, 64]
    emb = np.concatenate([freqs, freqs], axis=-1)       # [S, HD]
    cosT = np.cos(emb).T.astype(np.float32).copy()      # [HD, S]
    sinT = np.sin(emb).T.astype(np.float32)
    sinTs = np.concatenate([-sinT[:HD // 2], sinT[HD // 2:]], 0).astype(
        np.float32).copy()
    return cosT, sinTs


def _mask_diag():
    # maskD[p, i] = 1.0 iff p <= i (upper-triangular incl. diagonal)
    p = np.arange(128)[:, None]
    i = np.arange(128)[None, :]
    return (p <= i).astype(BF16)


def _tile_w_fm(w, n_tiles, kt):
    # [K, n_tiles*128] -> [n_tiles, 128(p of K), kt, 128]
    K, N = w.shape
    assert K == kt * 128 and N == n_tiles * 128
    return np.ascontiguousarray(
        w.reshape(kt, 128, n_tiles, 128).transpose(2, 1, 0, 3)).astype(BF16)


def _tile_w_tm(w, kt):
    # [K, N] -> [128(p of K), kt, N]
    K, N = w.shape
    assert K == kt * 128
    return np.ascontiguousarray(
        w.reshape(kt, 128, N).transpose(1, 0, 2)).astype(BF16)


def _tile_w_out(w):
    # [1024, D] -> [8(nb), 128(p of rows), 8(h), 512]
    return np.ascontiguousarray(
        w.reshape(8, 128, D // 512, 512).transpose(2, 1, 0, 3)).astype(BF16)


def _make_in_maps(hidden_states, w_q, w_k, w_v, w_o, w_lq, w_lk, w_lv, w_lo):
    cosT, sinTs = _rope_tables()
    maskD = _mask_diag()
    w2 = np.asarray(w_lq, dtype=np.float32) @ np.asarray(w_lk,
                                                         dtype=np.float32)
    in_maps = []
    for c in range(NCORES):
        b, g = divmod(c, TP)
        x = np.asarray(hidden_states[b], dtype=np.float32)       # [S, D]
        xt = np.ascontiguousarray(
            x.T.reshape(D_T, 128, S).transpose(1, 0, 2)).astype(BF16)
        qs = slice(g * HL * HD, (g + 1) * HL * HD)
        kvs = slice(g * KVL * HD, (g + 1) * KVL * HD)
        ls = slice(g * LHL * HD, (g + 1) * LHL * HD)
        wv_t = _tile_w_tm(np.asarray(w_v)[:, kvs], D_T)
        wlv_t = _tile_w_tm(np.asarray(w_lv)[:, ls], D_T)
        in_maps.append({
            "xt": xt,
            "wq": _tile_w_fm(np.asarray(w_q)[:, qs], HL, D_T),
            "wk": _tile_w_fm(np.asarray(w_k)[:, kvs], KVL, D_T),
            "wvc": np.ascontiguousarray(
                np.concatenate([wlv_t[:, :, 0:256], wv_t], axis=2)),
            "w2": _tile_w_fm(w2[:, ls], LHL, D_T),
            "wlvb": np.ascontiguousarray(wlv_t[:, :, 256:768]),
            "wlvc": np.ascontiguousarray(wlv_t[:, :, 768:1024]),
            "wo": _tile_w_out(np.asarray(w_o)[qs, :]),
            "wlo": _tile_w_out(np.asarray(w_lo)[ls, :]),
            "cosT": cosT,
            "sinTs": sinTs,
            "maskD": maskD,
        })
    return in_maps


def kernel(hidden_states, w_q, w_k, w_v, w_o, w_lq, w_lk, w_lv, w_lo):
    nc = _get_program()
    in_maps = _make_in_maps(hidden_states, w_q, w_k, w_v, w_o,
                            w_lq, w_lk, w_lv, w_lo)
    res = run_bass_kernel_spmd(nc, in_maps, list(range(NCORES))).results

    out = np.zeros((B, S, D), dtype=np.float32)
    for c in range(NCORES):
        b = c // TP
        out[b] += res[c]["out"]
    return out
